# revision 26
# baseline (speedup 1.0000x reference)
"""GRU + CRF NLL on 8 NeuronCores, optimized for axon wire cost + fp8 PE.

Per core (8 sequences): fp8 DoubleRow matmuls for the input projection and
the 512-step GRU recurrence; emissions and the CRF forward algorithm (exp
space, sigmoid-division trick, constant normalizer) run on-device
interleaved with the recurrence. Weights ship SHARDED (1/8 per core) and
are reassembled on-device with an AllGather, cutting wire bytes ~8x.
Outputs are tiny per-core CRF scalars; the host finishes the numerator
from tags and sums.
"""
import numpy as np

V, E, H, K, B, T = 32000, 512, 1024, 64, 64, 512
N_CORES = 8
BL = B // N_CORES          # 8 sequences per core
M0 = 4.2                   # CRF constant log-normalizer per step
RENORM = 32                # renorm cadence (steps)
NSLOT = 16                 # mbuf slots
SX, SW, SH = 16.0, 64.0, 16.0   # fp8 scales: x-embed, weights, hidden
SG = SX * SW               # = 1024: psum gate scale


def _to_bf16_u16(a):
    u = np.ascontiguousarray(np.asarray(a, np.float32)).view(np.uint32)
    u = u + 0x7FFF + ((u >> 16) & 1)
    return (u >> 16).astype(np.uint16)


_FP8_LUTS = {}


def _fp8_lut(scale):
    """u16 (bf16 bits) -> u8 bits of float8_e4m3(value*scale); scale is a
    power of two so the fold is exact."""
    if scale not in _FP8_LUTS:
        import ml_dtypes
        allu = np.arange(65536, dtype=np.uint16)
        f = allu.view(ml_dtypes.bfloat16).astype(np.float32) * scale
        f = np.clip(f, -240.0, 240.0)
        with np.errstate(invalid="ignore"):
            _FP8_LUTS[scale] = f.astype(ml_dtypes.float8_e4m3).view(np.uint8)
    return _FP8_LUTS[scale]


def _to_fp8_u8(a, scale):
    u = np.ascontiguousarray(np.asarray(a, np.float32)).view(np.uint32)
    idx = (u + 0x7FFF + ((u >> 16) & 1)) >> 16
    return np.take(_fp8_lut(scale), idx)


def _patch_tile_wait_split():
    from concourse import tile as _tile
    import concourse.mybir as mybir

    cls = None
    for obj in vars(_tile).values():
        if isinstance(obj, type) and "_commit_instruction" in vars(obj):
            cls = obj
            break
    if cls is None or getattr(cls, "_wait_split_patched", False):
        return
    orig = cls._commit_instruction
    ET = mybir.EngineType
    compute = {ET.PE, ET.DVE, ET.Activation, ET.Pool, ET.SP}

    def wrapper(self, inst, lazy_reg_writes=True):
        si = getattr(inst, "sync_info", None)
        eng = getattr(inst, "engine", None)
        if (si is not None and si.on_wait and len(si.on_wait) > 1
                and eng in compute and not isinstance(inst, mybir.InstNoOp)):
            waits = list(si.on_wait)
            for w in waits[:-1]:
                nop = mybir.InstNoOp(
                    name=self.nc.get_next_instruction_name(),
                    engine=eng, bass_nofuse=True,
                    sync_info=mybir.SyncInfo(on_wait=[w], on_update=[]))
                orig(self, nop, lazy_reg_writes)
            inst.sync_info = mybir.SyncInfo(
                on_wait=[waits[-1]], on_update=si.on_update)
        return orig(self, inst, lazy_reg_writes)

    cls._commit_instruction = wrapper

    if "_drain_and_barrier" in vars(cls):
        SC = _tile.ScopedClock

        def patched_db(self, tick_clock, wait_clock):
            drain_inst = self.nc.sync.drain()
            wait_clock.add_sem_waits(
                drain_inst.ins, SC({None: tick_clock.global_clock}))
            d = drain_inst.ins
            si = getattr(d, "sync_info", None)
            if si is not None and si.on_wait and len(si.on_wait) > 1:
                waits = list(si.on_wait)
                d.sync_info = mybir.SyncInfo(
                    on_wait=waits[:1], on_update=si.on_update or [])
                for w in waits[1:]:
                    nop = mybir.InstNoOp(
                        name=self.nc.get_next_instruction_name(),
                        engine=ET.SP, bass_nofuse=True,
                        sync_info=mybir.SyncInfo(on_wait=[w], on_update=[]))
                    self.nc.sync.add_instruction(nop)
            self.nc.all_engine_barrier()
            assert self.sems is not None
            popped = self.nc._tile_sem_poison_stack.pop()
            assert popped is self._sem_poison
            self.nc.clear_and_free_semaphores(
                list(self.sems.allocated().values()))
            self.nc.all_engine_barrier()

        cls._drain_and_barrier = patched_db
    cls._wait_split_patched = True


def _build_nc(t_steps, shard_weights=True):
    import concourse.bass as bass
    import concourse.mybir as mybir
    from concourse.tile import TileContext

    _patch_tile_wait_split()

    f32 = mybir.dt.float32
    bf16 = mybir.dt.bfloat16
    f8 = mybir.dt.float8e4
    AF = mybir.ActivationFunctionType
    ALU = mybir.AluOpType
    DR = mybir.MatmulPerfMode.DoubleRow
    TOK = BL * t_steps
    NTILE = max(1, TOK // 128)
    H3 = 3 * H

    nc = bass.Bass(num_devices=N_CORES)
    # ---- inputs ----
    xe8 = nc.declare_dram_parameter("xe8", [TOK, E], f8, isOutput=False)
    if shard_weights:
        whh_in = nc.declare_dram_parameter("whh8", [H // 8, H3], f8,
                                           isOutput=False)
        wih_in = nc.declare_dram_parameter("wih8", [E // 8, H3], f8,
                                           isOutput=False)
        fcw_in = nc.declare_dram_parameter("fcw8", [H // 8, K], f8,
                                           isOutput=False)
        whh_d = nc.dram_tensor("whh_g", [H, H3], f8, kind="Internal")
        wih_d = nc.dram_tensor("wih_g", [E, H3], f8, kind="Internal")
        fcw_d = nc.dram_tensor("fcw_g", [H, K], f8, kind="Internal")
        whh_s = nc.dram_tensor("whh_s", [H // 8, H3], f8, kind="Internal")
        wih_s = nc.dram_tensor("wih_s", [E // 8, H3], f8, kind="Internal")
        fcw_s = nc.dram_tensor("fcw_s", [H // 8, K], f8, kind="Internal")
    else:
        whh_d = nc.declare_dram_parameter("whh8", [H, H3], f8, isOutput=False)
        wih_d = nc.declare_dram_parameter("wih8", [E, H3], f8, isOutput=False)
        fcw_d = nc.declare_dram_parameter("fcw8", [H, K], f8, isOutput=False)
    biasrow = nc.declare_dram_parameter("biasrow", [1, H3], bf16,
                                        isOutput=False)
    bhnrow = nc.declare_dram_parameter("bhnrow", [1, H], bf16, isOutput=False)
    tagrow = nc.declare_dram_parameter("tagrow", [1, TOK], bf16,
                                       isOutput=False)
    expT_in = nc.declare_dram_parameter("expT", [K, K], bf16, isOutput=False)
    crfb0_in = nc.declare_dram_parameter("crfb0", [K, 1], f32, isOutput=False)
    ncrfb0_in = nc.declare_dram_parameter("ncrfb0", [K, 1], f32,
                                          isOutput=False)
    crfb_in = nc.declare_dram_parameter("crfb", [K, 1], f32, isOutput=False)
    ncrfb_in = nc.declare_dram_parameter("ncrfb", [K, 1], f32, isOutput=False)
    expend_in = nc.declare_dram_parameter("expend", [K, 1], f32,
                                          isOutput=False)
    id128_in = nc.declare_dram_parameter("id128", [128, 128], f8,
                                         isOutput=False)
    id8f8_in = nc.declare_dram_parameter("id8f8", [BL, BL], f8, isOutput=False)
    id8bf_in = nc.declare_dram_parameter("id8bf", [BL, BL], bf16,
                                         isOutput=False)
    id64bf_in = nc.declare_dram_parameter("id64bf", [K, K], bf16,
                                          isOutput=False)
    id16bf_in = nc.declare_dram_parameter("id16bf", [BL, 2 * BL], bf16,
                                          isOutput=False)
    # ---- outputs ----
    emgold_o = nc.declare_dram_parameter("emgold", [K, BL], f32, isOutput=True)
    finsum_o = nc.declare_dram_parameter("finsum", [1, BL], f32, isOutput=True)
    mbuf_o = nc.declare_dram_parameter("mbuf", [BL, NSLOT], f32, isOutput=True)
    gi_d = nc.dram_tensor("gi_scratch", [TOK, H3], bf16, kind="Internal")

    if shard_weights:
        cc_sem = nc.alloc_semaphore("cc_sem")
        cp_sem = nc.alloc_semaphore("cp_sem")
        groups = [list(range(N_CORES))]
        for src, stg in ((whh_in, whh_s), (wih_in, wih_s),
                         (fcw_in, fcw_s)):
            nc.sync.dma_start(out=stg[:], in_=src[:]).then_inc(cp_sem, 16)
        nc.gpsimd.wait_ge(cp_sem, 48)
        for stg, dst in ((whh_s, whh_d), (wih_s, wih_d),
                         (fcw_s, fcw_d)):
            nc.gpsimd.collective_compute(
                "AllGather", mybir.AluOpType.bypass,
                replica_groups=groups,
                ins=[stg[:].opt()], outs=[dst[:].opt()]).then_inc(cc_sem)
        nc.sync.wait_ge(cc_sem, 3)

    with TileContext(nc) as tc:
        with (
            tc.tile_pool(name="wpool", bufs=1) as wpool,
            tc.tile_pool(name="iopool", bufs=4) as iopool,
            tc.tile_pool(name="gates", bufs=2) as gpool,
            tc.tile_pool(name="hpool", bufs=2) as hpool,
            tc.tile_pool(name="crfpool", bufs=2) as cpool,
        ):
            # ---------------- prelude: consts ----------------
            ones_row = wpool.tile([1, 128], bf16, tag="ones")
            nc.vector.memset(ones_row[:], 1.0)
            onescol = wpool.tile([K, 2], bf16, tag="onescol")
            nc.vector.memset(onescol[:], 1.0)
            mbuf_sb = wpool.tile([BL, NSLOT], f32, tag="mbuf")
            nc.vector.memset(mbuf_sb[:], 1.0)
            iota_col = wpool.tile([K, BL], bf16, tag="iotac")
            nc.gpsimd.iota(iota_col[:], pattern=[[0, BL]], base=0,
                           channel_multiplier=1,
                           allow_small_or_imprecise_dtypes=True)

            expT_sb = wpool.tile([K, K], bf16, tag="expT")
            nc.sync.dma_start(out=expT_sb[:], in_=expT_in[:])
            crfb0 = wpool.tile([K, 1], f32, tag="crfb0")
            nc.sync.dma_start(out=crfb0[:], in_=crfb0_in[:])
            ncrfb0 = wpool.tile([K, 1], f32, tag="ncrfb0")
            nc.sync.dma_start(out=ncrfb0[:], in_=ncrfb0_in[:])
            crfb = wpool.tile([K, 1], f32, tag="crfb")
            nc.sync.dma_start(out=crfb[:], in_=crfb_in[:])
            ncrfb = wpool.tile([K, 1], f32, tag="ncrfb")
            nc.sync.dma_start(out=ncrfb[:], in_=ncrfb_in[:])
            expend = wpool.tile([K, 1], f32, tag="expend")
            nc.sync.dma_start(out=expend[:], in_=expend_in[:])
            id128 = wpool.tile([128, 128], f8, tag="id128")
            nc.sync.dma_start(out=id128[:], in_=id128_in[:])
            id8f8 = wpool.tile([BL, BL], f8, tag="id8f8")
            nc.sync.dma_start(out=id8f8[:], in_=id8f8_in[:])
            id8bf = wpool.tile([BL, BL], bf16, tag="id8bf")
            nc.sync.dma_start(out=id8bf[:], in_=id8bf_in[:])
            id64bf = wpool.tile([K, K], bf16, tag="id64bf")
            nc.sync.dma_start(out=id64bf[:], in_=id64bf_in[:])
            id16bf = wpool.tile([BL, 2 * BL], bf16, tag="id16bf")
            nc.sync.dma_start(out=id16bf[:], in_=id16bf_in[:])
            brow_sb = wpool.tile([1, H3], bf16, tag="brow")
            nc.sync.dma_start(out=brow_sb[:], in_=biasrow[:])
            bhn_sb = wpool.tile([1, H], bf16, tag="bhnrow")
            nc.sync.dma_start(out=bhn_sb[:], in_=bhnrow[:])
            tagrow_sb = wpool.tile([1, TOK], bf16, tag="tagrow")
            nc.sync.dma_start(out=tagrow_sb[:], in_=tagrow[:])

            bias_sb = wpool.tile([128, H3], bf16, tag="biasb")
            tags_sb = wpool.tile([K, TOK], bf16, tag="tagsb")
            with tc.tile_pool(name="prelps", bufs=1, space="PSUM") as prelps:
                # broadcast bias row -> [128, 3H]
                for c in range(H3 // 512):
                    bps = prelps.tile([128, 512], f32, tag="bps")
                    nc.tensor.matmul(bps[:], ones_row[:],
                                     brow_sb[:, c * 512:(c + 1) * 512],
                                     start=True, stop=True)
                    nc.scalar.copy(bias_sb[:, c * 512:(c + 1) * 512], bps[:])
                # broadcast tag row -> [64, TOK]
                for c in range((TOK + 511) // 512):
                    w = min(512, TOK - c * 512)
                    tps = prelps.tile([K, 512], f32, tag="tps")
                    nc.tensor.matmul(tps[:, 0:w], ones_row[:, 0:K],
                                     tagrow_sb[:, c * 512:c * 512 + w],
                                     start=True, stop=True)
                    nc.scalar.copy(tags_sb[:, c * 512:c * 512 + w],
                                   tps[:, 0:w])

            # ---------------- weights (after collectives) ----------------
            whh_sb = wpool.tile([128, 8 * H3], f8, tag="whh")
            nc.sync.dma_start(
                out=whh_sb[:].rearrange("p (k g) -> p k g", k=8),
                in_=whh_d.rearrange("(k p) g -> p k g", p=128))
            wih_sb = wpool.tile([128, 4 * H3], f8, tag="wih")
            nc.sync.dma_start(
                out=wih_sb[:].rearrange("p (k g) -> p k g", k=4),
                in_=wih_d.rearrange("(k p) g -> p k g", p=128))
            fcw_sb = wpool.tile([128, 8 * K], f8, tag="fcw")
            nc.sync.dma_start(
                out=fcw_sb[:].rearrange("p (k j) -> p k j", k=8),
                in_=fcw_d.rearrange("(k p) j -> p k j", p=128))

            # ---------------- phase A: gi -> gi_d ----------------
            with (
                tc.tile_pool(name="apool", bufs=3) as apool,
                tc.tile_pool(name="apsA", bufs=2, space="PSUM") as apsA,
                tc.tile_pool(name="apsT", bufs=2, space="PSUM") as apsT,
            ):
                wih3 = wih_sb[:].rearrange("p (k g) -> p k g", k=4)
                for tt in range(NTILE):
                    xs = apool.tile([128, E], f8, tag="xs")
                    nc.sync.dma_start(out=xs[:],
                                      in_=xe8[tt * 128:(tt + 1) * 128, :])
                    xt_ps = apsT.tile([128, 2 * E], f8, tag="xtp")
                    xt_ps2 = xt_ps[:].rearrange("p (e two) -> p e two", two=2)
                    for ec in range(4):
                        nc.tensor.transpose(
                            xt_ps2[:, ec * 128:(ec + 1) * 128, 0:1],
                            xs[:, ec * 128:(ec + 1) * 128], id128[:])
                    xt = apool.tile([128, E], f8, tag="xt")
                    nc.vector.tensor_copy(xt[:], xt_ps2[:, :, 0:1])
                    xt3 = xt[:].rearrange("p (k e) -> p k e", k=4)
                    for gc in range(H3 // 512):
                        ps = apsA.tile([128, 512], f32, tag="aps")
                        for kp in range(2):
                            nc.tensor.matmul(
                                ps[:],
                                xt3[:, 2 * kp:2 * kp + 2, :],
                                wih3[:, 2 * kp:2 * kp + 2,
                                     gc * 512:gc * 512 + 512],
                                start=(kp == 0), stop=(kp == 1),
                                perf_mode=DR)
                        gi_sb = apool.tile([128, 512], bf16, tag="gia")
                        nc.vector.tensor_tensor(
                            gi_sb[:], ps[:],
                            bias_sb[:, gc * 512:gc * 512 + 512], op=ALU.add)
                        nc.sync.dma_start(
                            out=gi_d[tt * 128:(tt + 1) * 128,
                                     gc * 512:gc * 512 + 512],
                            in_=gi_sb[:])

            # ---------------- main loop ----------------
            with (
                tc.tile_pool(name="psG", bufs=1, space="PSUM") as psG,
                tc.tile_pool(name="psHT", bufs=1, space="PSUM") as psHT,
                tc.tile_pool(name="psEC", bufs=2, space="PSUM") as psEC,
                tc.tile_pool(name="psMS", bufs=1, space="PSUM") as psMS,
                tc.tile_pool(name="psMB", bufs=1, space="PSUM") as psMB,
            ):
                whh3 = whh_sb[:].rearrange("p (k g) -> p k g", k=8)
                fcw3 = fcw_sb[:].rearrange("p (k j) -> p k j", k=8)

                ms_tile = psMS.tile([K, 128], f32, tag="ms")
                mb_tile = psMB.tile([BL, 128], bf16, tag="mb")
                tr_ps = psHT.tile([128, 128], bf16, tag="trp")
                hT8 = hpool.tile([128, 128], f8, tag="hT8")
                nc.vector.memset(hT8[:], 0.0)
                h_sb = hpool.tile([BL, H], bf16, tag="h")
                nc.vector.memset(h_sb[:], 0.0)
                aT = cpool.tile([K, BL], bf16, tag="aT")
                acc = wpool.tile([K, BL], f32, tag="acc")
                nc.vector.memset(acc[:], 0.0)

                def em_crf(v, hT8_v, aT_prev):
                    """emissions + CRF for step v (hT8_v = hidden after v)."""
                    hT3 = hT8_v[:].rearrange("p (k b) -> p k b", k=8)
                    ec = psEC.tile([K, 2 * BL], f32, tag="ec")
                    em_ps = ec[:, 0:BL]
                    for kp in range(4):
                        nc.tensor.matmul(
                            em_ps, fcw3[:, 2 * kp:2 * kp + 2, :],
                            hT3[:, 2 * kp:2 * kp + 2, 0:BL],
                            start=(kp == 0), stop=(kp == 3), perf_mode=DR)
                    first = (v == 0)
                    sp = cpool.tile([K, BL], f32, tag="sp")
                    nc.scalar.activation(sp[:], em_ps, AF.Sigmoid,
                                         bias=(crfb0 if first else crfb)[:],
                                         scale=1.0 / SG)
                    sm = cpool.tile([K, BL], f32, tag="sm")
                    nc.scalar.activation(sm[:], em_ps, AF.Sigmoid,
                                         bias=(ncrfb0 if first else ncrfb)[:],
                                         scale=-1.0 / SG)
                    smr = cpool.tile([K, BL], f32, tag="smr")
                    nc.vector.reciprocal(smr[:], sm[:])
                    eem = cpool.tile([K, BL], f32, tag="eem")
                    nc.vector.tensor_tensor(eem[:], sp[:], smr[:],
                                            op=ALU.mult)
                    if first:
                        aT_new = cpool.tile([K, BL], bf16, tag="aT")
                        nc.vector.tensor_copy(aT_new[:], eem[:])
                    else:
                        crf_ps = ec[:, BL:2 * BL]
                        nc.tensor.matmul(crf_ps, expT_sb[:], aT_prev[:],
                                         start=True, stop=True)
                        aT_new = cpool.tile([K, BL], bf16, tag="aT")
                        nc.vector.tensor_tensor(aT_new[:], crf_ps, eem[:],
                                                op=ALU.mult)
                    # gold emission accumulation
                    oh = cpool.tile([K, BL], bf16, tag="oh")
                    nc.vector.tensor_tensor(
                        oh[:], tags_sb[:, v * BL:(v + 1) * BL], iota_col[:],
                        op=ALU.is_equal)
                    gold = cpool.tile([K, BL], f32, tag="gold")
                    nc.vector.tensor_tensor(gold[:], em_ps, oh[:],
                                            op=ALU.mult)
                    nc.vector.tensor_tensor(acc[:], acc[:], gold[:],
                                            op=ALU.add)
                    # periodic renorm
                    if (not first) and v % RENORM == 0:
                        slot = v // RENORM
                        rps = mb_tile[0:BL, 0:K]
                        nc.tensor.transpose(rps, aT_new[:], id64bf[:])
                        m = cpool.tile([BL, 1], f32, tag="m")
                        nc.vector.tensor_reduce(m[:], rps,
                                                axis=mybir.AxisListType.X,
                                                op=ALU.max)
                        rcpf = cpool.tile([BL, 1], f32, tag="rcpf")
                        nc.vector.reciprocal(rcpf[:], m[:])
                        rcp = cpool.tile([BL, 2], bf16, tag="rcp")
                        nc.vector.tensor_copy(rcp[:, 0:1], rcpf[:])
                        nc.vector.tensor_copy(rcp[:, 1:2], rcpf[:])
                        nc.vector.tensor_copy(mbuf_sb[:, slot:slot + 1],
                                              rcp[:, 0:1])
                        rps2 = mb_tile[0:2, K:K + BL]
                        nc.tensor.transpose(rps2, rcp[:], id8bf[:])
                        rrow = cpool.tile([1, BL], bf16, tag="rrow")
                        nc.scalar.copy(rrow[:], rps2[0:1, :])
                        rb_ps = ms_tile[0:K, 0:BL]
                        nc.tensor.matmul(rb_ps, ones_row[:, 0:K], rrow[:],
                                         start=True, stop=True)
                        aT2 = cpool.tile([K, BL], bf16, tag="aT")
                        nc.vector.tensor_tensor(aT2[:], aT_new[:], rb_ps,
                                                op=ALU.mult)
                        aT_new = aT2
                    return aT_new

                for t in range(t_steps):
                    # emissions + CRF for the previous step (hidden ready)
                    if t > 0:
                        aT = em_crf(t - 1, hT8, aT)
                    gi_t = iopool.tile([BL, H3], bf16, tag="gib")
                    nc.sync.dma_start(out=gi_t[:],
                                      in_=gi_d[t * BL:(t + 1) * BL, :])
                    # gh chunks: q0,q1=r; q2,q3=z; q4,q5=n
                    hT3 = hT8[:].rearrange("p (k b) -> p k b", k=8)
                    gq = []
                    for q in range(6):
                        g16 = psG.tile([2 * BL, 512], f32, tag="g%d" % (q % 3))
                        for kp in range(4):
                            nc.tensor.matmul(
                                g16[:], hT3[:, 2 * kp:2 * kp + 2, :],
                                whh3[:, 2 * kp:2 * kp + 2,
                                     q * 512:q * 512 + 512],
                                start=(kp == 0), stop=False, perf_mode=DR)
                        if q < 4:
                            nc.tensor.matmul(
                                g16[:], id16bf[:],
                                gi_t[:, q * 512:q * 512 + 512],
                                start=False, stop=True)
                        else:
                            nc.tensor.matmul(
                                g16[:], ones_row[:, 0:2 * BL],
                                bhn_sb[:, (q - 4) * 512:(q - 4) * 512 + 512],
                                start=False, stop=True)
                        gq.append(g16[0:BL, :])
                    rz = gpool.tile([BL, 2 * H], bf16, tag="rz")
                    for q in range(4):
                        nc.scalar.activation(rz[:, q * 512:q * 512 + 512],
                                             gq[q][:], AF.Sigmoid,
                                             scale=1.0 / SG)
                    n_sb = gpool.tile([BL, H], bf16, tag="n")
                    hn = hpool.tile([BL, H], bf16, tag="h")
                    if t == 0:
                        for jj in range(8):
                            nc.tensor.transpose(
                                tr_ps[:, jj * 16 + 8:jj * 16 + 16],
                                h_sb[:, jj * 128:(jj + 1) * 128], id8bf[:])
                    for c in range(2):
                        sl = slice(c * 512, c * 512 + 512)
                        t1 = gpool.tile([BL, 512], bf16, tag="t1%d" % c)
                        nc.vector.tensor_tensor(t1[:], gq[4 + c][:],
                                                rz[:, sl], op=ALU.mult)
                        t2 = gpool.tile([BL, 512], bf16, tag="t2%d" % c)
                        nc.vector.tensor_tensor(
                            t2[:], t1[:], gi_t[:, 2 * H + c * 512:
                                               2 * H + c * 512 + 512],
                            op=ALU.add)
                        nc.scalar.activation(n_sb[:, sl], t2[:], AF.Tanh,
                                             scale=1.0 / SG)
                        s_c = gpool.tile([BL, 512], bf16, tag="s%d" % c)
                        nc.vector.tensor_tensor(s_c[:], h_sb[:, sl],
                                                n_sb[:, sl], op=ALU.subtract)
                        p_c = gpool.tile([BL, 512], bf16, tag="p%d" % c)
                        nc.vector.tensor_tensor(p_c[:], rz[:, H + sl.start:
                                                           H + sl.stop],
                                                s_c[:], op=ALU.mult)
                        nc.vector.tensor_tensor(hn[:, sl], n_sb[:, sl],
                                                p_c[:], op=ALU.add)
                        for j in range(4):
                            jj = c * 4 + j
                            nc.tensor.transpose(
                                tr_ps[:, jj * 16:jj * 16 + 8],
                                hn[:, jj * 128:(jj + 1) * 128], id8bf[:])
                    h_sb = hn
                    hT8 = hpool.tile([128, 128], f8, tag="hT8")
                    nc.scalar.activation(hT8[:], tr_ps[:], AF.Copy, scale=SH)

                # ---------------- epilogue ----------------
                aT = em_crf(t_steps - 1, hT8, aT)
                fin = cpool.tile([K, BL], bf16, tag="fin")
                nc.vector.tensor_scalar_mul(fin[:], aT[:], expend[:])
                fs_ps = ms_tile[0:2, BL:2 * BL]
                nc.tensor.matmul(fs_ps, onescol[:], fin[:],
                                 start=True, stop=True)
                fs_sb = cpool.tile([1, BL], f32, tag="fssb")
                nc.scalar.copy(fs_sb[:], fs_ps[0:1, :])
                nc.sync.dma_start(out=finsum_o[:], in_=fs_sb[:])
                nc.sync.dma_start(out=emgold_o[:], in_=acc[:])
                nc.sync.dma_start(out=mbuf_o[:], in_=mbuf_sb[:])
    return nc


_NC_CACHE = {}


class _NcShim:
    """Duck-typed stand-in for Bass in run_bass_via_pjrt + lowering: needs
    .m, .to_json_bytes(), .has_collectives, .dbg_addr, .partition_id_tensor.
    """

    def __init__(self, json_bytes):
        import types
        import concourse.mybir as mybir
        self.m = mybir.module_from_json_bytes(json_bytes)
        self._json = json_bytes
        self.has_collectives = True
        self.dbg_addr = None
        self.target_bir_lowering = False
        self.partition_id_tensor = None
        for alloc in self.m.functions[0].allocations:
            if not isinstance(alloc, mybir.MemoryLocationSet):
                continue
            if (alloc.kind == "ExternalInput"
                    and alloc.memorylocations
                    and alloc.memorylocations[0].name == "partition_id"):
                self.partition_id_tensor = types.SimpleNamespace(
                    name="partition_id")

    def to_json_bytes(self):
        return self._json


def _build_version():
    import hashlib
    import inspect
    src = inspect.getsource(_build_nc)
    return hashlib.sha256(src.encode()).hexdigest()[:16]


def _get_nc(t_steps, shard):
    import os
    key = (t_steps, shard)
    if key in _NC_CACHE:
        return _NC_CACHE[key]
    path = os.path.join(_NEFF_CACHE_DIR, "bir_%s_%s_%s.json"
                        % (t_steps, int(shard), _build_version()))
    if os.path.exists(path):
        with open(path, "rb") as f:
            nc = _NcShim(f.read())
    else:
        nc = _build_nc(t_steps, shard)
        try:
            os.makedirs(_NEFF_CACHE_DIR, exist_ok=True)
            data = nc.to_json_bytes()
            tmp = path + ".tmp.%d" % os.getpid()
            with open(tmp, "wb") as f:
                f.write(data)
            os.replace(tmp, path)
        except Exception:
            pass
    _NC_CACHE[key] = nc
    return _NC_CACHE[key]


def _make_in_maps(x, tags, emb, w_ih, w_hh, b_ih, b_hh, fc_w, fc_b,
                  start_trans, end_trans, trans, t_steps=T, shard=True):
    import ml_dtypes
    as8 = lambda u: u.view(ml_dtypes.float8_e4m3)
    asbf = lambda u: u.view(ml_dtypes.bfloat16)

    emb8 = _to_fp8_u8(emb, SX)                      # [V, E]
    whh8 = _to_fp8_u8(np.ascontiguousarray(w_hh.T), SW)   # [H, 3H]
    wih8 = _to_fp8_u8(np.ascontiguousarray(w_ih.T), SW)   # [E, 3H]
    fcw8 = _to_fp8_u8(np.ascontiguousarray(fc_w.T), SW)   # [H, K]
    biasrow = np.concatenate([(b_ih[:2 * H] + b_hh[:2 * H]),
                              b_ih[2 * H:]])[None, :] * SG
    bhnrow = (b_hh[None, 2 * H:] * SG)
    expT = _to_bf16_u16(np.exp(trans))
    crfb0 = (fc_b + start_trans - M0).astype(np.float32)[:, None]
    crfb = (fc_b - M0).astype(np.float32)[:, None]
    expend = np.exp(end_trans).astype(np.float32)[:, None]
    id128 = _to_fp8_u8(np.eye(128, dtype=np.float32), 1.0)
    id8f8 = _to_fp8_u8(np.eye(BL, dtype=np.float32), 1.0)
    id8bf = _to_bf16_u16(np.eye(BL, dtype=np.float32))
    id64bf = _to_bf16_u16(np.eye(K, dtype=np.float32))
    id16bf = _to_bf16_u16(np.concatenate([np.eye(BL), np.eye(BL)],
                                         axis=1).astype(np.float32))

    in_maps = []
    for c in range(N_CORES):
        idxT = x[c * BL:(c + 1) * BL, :t_steps].T.ravel()
        xe8 = np.take(emb8, idxT, axis=0)           # [TOK, E] u8
        tagT = tags[c * BL:(c + 1) * BL, :t_steps].T.reshape(1, -1)
        m = {
            "xe8": as8(xe8),
            "biasrow": asbf(_to_bf16_u16(biasrow)),
            "bhnrow": asbf(_to_bf16_u16(bhnrow)),
            "tagrow": asbf(_to_bf16_u16(tagT.astype(np.float32))),
            "expT": asbf(expT),
            "crfb0": crfb0, "ncrfb0": -crfb0,
            "crfb": crfb, "ncrfb": -crfb,
            "expend": expend,
            "id128": as8(id128), "id8f8": as8(id8f8),
            "id8bf": asbf(id8bf), "id64bf": asbf(id64bf),
            "id16bf": asbf(id16bf),
        }
        if shard:
            m["whh8"] = as8(whh8[c * (H // 8):(c + 1) * (H // 8)])
            m["wih8"] = as8(wih8[c * (E // 8):(c + 1) * (E // 8)])
            m["fcw8"] = as8(fcw8[c * (H // 8):(c + 1) * (H // 8)])
        else:
            m["whh8"] = as8(whh8)
            m["wih8"] = as8(wih8)
            m["fcw8"] = as8(fcw8)
        in_maps.append(m)
    return in_maps


def _finish_host(res, tags, fc_b, start_trans, end_trans, trans, t_steps=T):
    nll = 0.0
    for c in range(N_CORES):
        emgold = np.asarray(res[c]["emgold"], np.float32)    # [K, BL]
        finsum = np.asarray(res[c]["finsum"], np.float32)[0]  # [BL]
        mbuf = np.asarray(res[c]["mbuf"], np.float32)         # [BL, NSLOT]
        tg = tags[c * BL:(c + 1) * BL, :t_steps]
        den = (np.log(finsum) - np.log(mbuf).sum(axis=1)
               + M0 * t_steps)
        emg = emgold.sum(axis=0) / SG + np.take(fc_b, tg).sum(axis=1)
        num = start_trans[tg[:, 0]] + emg
        num += trans[tg[:, :-1], tg[:, 1:]].sum(axis=1)
        num += end_trans[tg[:, -1]]
        nll += float((den - num).sum())
    return nll


_NEFF_CACHE_DIR = "/root/.cache/bass_neff_cache"


def _install_neff_cache():
    """Disk-cache the HLO->NEFF compile (walrus takes 10-80s per fresh
    process otherwise; the stock path has no persistent cache here)."""
    import concourse.bass2jax as b2j
    if getattr(b2j, "_neff_cache_installed", False):
        return
    import hashlib
    import os
    orig = b2j.neuronx_cc_hook

    def cached_hook(code, code_format, platform_version, file_prefix):
        if b"bass_exec" not in code:
            return orig(code, code_format, platform_version, file_prefix)
        key = hashlib.sha256(code).hexdigest()
        path = os.path.join(_NEFF_CACHE_DIR, key + ".bin")
        if os.path.exists(path):
            with open(path, "rb") as f:
                return 0, f.read()
        ret, data = orig(code, code_format, platform_version, file_prefix)
        if ret == 0 and isinstance(data, (bytes, bytearray)):
            os.makedirs(_NEFF_CACHE_DIR, exist_ok=True)
            tmp = path + ".tmp.%d" % os.getpid()
            with open(tmp, "wb") as f:
                f.write(data)
            os.replace(tmp, path)
        return ret, data

    # Second-level cache keyed on the BIR json itself: the serialized HLO
    # bytes are not deterministic across processes, so the whole-result
    # cache above can miss; the BIR is stable and the walrus compile is
    # the expensive part (the per-variant tensor rename is cheap).
    orig_cbk = b2j.compile_bir_kernel

    def cached_cbk(bir_json, tmpdir, neff_name="file.neff"):
        data = bir_json if isinstance(bir_json, bytes) else bir_json.encode()
        key = hashlib.sha256(data).hexdigest()
        path = os.path.join(_NEFF_CACHE_DIR, key + ".neff")
        out_path = os.path.join(tmpdir, neff_name)
        if os.path.exists(path):
            import shutil
            shutil.copy(path, out_path)
            return out_path
        neff_file = orig_cbk(bir_json, tmpdir, neff_name)
        try:
            os.makedirs(_NEFF_CACHE_DIR, exist_ok=True)
            tmp = path + ".tmp.%d" % os.getpid()
            import shutil
            shutil.copy(neff_file, tmp)
            os.replace(tmp, path)
        except Exception:
            pass
        return neff_file

    b2j.compile_bir_kernel = cached_cbk
    b2j.neuronx_cc_hook = cached_hook
    b2j._neff_cache_installed = True


def _run_spmd_fast(nc, in_maps):
    """run_bass_via_pjrt equivalent, but inputs are pre-placed on the mesh
    with sharded device_put (~10 ms/MB) instead of the jit-argument
    transfer path (~75 ms/MB)."""
    import jax
    import concourse.mybir as mybir
    from jax.sharding import Mesh, PartitionSpec, NamedSharding
    from jax.experimental.shard_map import shard_map
    from concourse import bass2jax as b2j

    b2j.install_neuronx_cc_hook()
    partition_name = (nc.partition_id_tensor.name
                      if nc.partition_id_tensor else None)
    in_names, out_names, out_avals, zero_outs = [], [], [], []
    for alloc in nc.m.functions[0].allocations:
        if not isinstance(alloc, mybir.MemoryLocationSet):
            continue
        name = alloc.memorylocations[0].name
        if alloc.kind == "ExternalInput":
            if name != partition_name:
                in_names.append(name)
        elif alloc.kind == "ExternalOutput":
            shape = tuple(alloc.tensor_shape)
            dtype = mybir.dt.np(alloc.dtype)
            out_names.append(name)
            out_avals.append(jax.core.ShapedArray(shape, dtype))
            zero_outs.append(np.zeros(shape, dtype))
    n_params = len(in_names)
    n_outs = len(out_avals)
    all_in_names = list(in_names) + list(out_names)
    if partition_name is not None:
        all_in_names.append(partition_name)

    devices = jax.devices()[:N_CORES]
    mesh = Mesh(np.asarray(devices), ("core",))
    sh = NamedSharding(mesh, PartitionSpec("core"))
    placed = []
    for i, name in enumerate(in_names):
        g = np.concatenate([in_maps[c][name] for c in range(N_CORES)], axis=0)
        placed.append(jax.device_put(g, sh))   # async
    concat_zeros = [np.zeros((N_CORES * z.shape[0], *z.shape[1:]), z.dtype)
                    for z in zero_outs]

    def _body(*args):
        operands = list(args)
        if partition_name is not None:
            operands.append(b2j.partition_id_tensor())
        outs = b2j._bass_exec_p.bind(
            *operands,
            out_avals=tuple(out_avals),
            in_names=tuple(all_in_names),
            out_names=tuple(out_names),
            lowering_input_output_aliases=(),
            sim_require_finite=True,
            sim_require_nnan=True,
            nc=nc,
        )
        return tuple(outs)

    donate = tuple(range(n_params, n_params + n_outs))
    sharded = jax.jit(
        shard_map(_body, mesh=mesh,
                  in_specs=(PartitionSpec("core"),) * (n_params + n_outs),
                  out_specs=(PartitionSpec("core"),) * n_outs,
                  check_rep=False),
        donate_argnums=donate, keep_unused=True)
    out_arrs = sharded(*placed, *concat_zeros)
    return [
        {name: np.asarray(out_arrs[i]).reshape(
            N_CORES, *out_avals[i].shape)[c]
         for i, name in enumerate(out_names)}
        for c in range(N_CORES)
    ]


def _run_device(inputs, t_steps=T, shard=True):
    import sys
    import time as _time
    from concourse.bass_utils import run_bass_kernel_spmd
    _install_neff_cache()
    t0 = _time.time()
    nc = _get_nc(t_steps, shard)
    t1 = _time.time()
    in_maps = _make_in_maps(**inputs, t_steps=t_steps, shard=shard)
    t2 = _time.time()
    try:
        res = _run_spmd_fast(nc, in_maps)
    except Exception:
        import traceback
        traceback.print_exc()
        res = run_bass_kernel_spmd(nc, in_maps,
                                   list(range(N_CORES))).results
    t3 = _time.time()
    out = _finish_host(res, inputs["tags"], inputs["fc_b"],
                       inputs["start_trans"], inputs["end_trans"],
                       inputs["trans"], t_steps=t_steps)
    t4 = _time.time()
    print("[kernel] build=%.0fms host_prep=%.0fms device=%.0fms "
          "finish=%.0fms" % ((t1 - t0) * 1e3, (t2 - t1) * 1e3,
                             (t3 - t2) * 1e3, (t4 - t3) * 1e3),
          file=sys.stderr)
    return out


def _host_fallback(x, tags, emb, w_ih, w_hh, b_ih, b_hh, fc_w, fc_b,
                   start_trans, end_trans, trans):
    xe = emb[x]
    gi = (xe.reshape(-1, E) @ w_ih.T + b_ih).reshape(B, T, 3 * H)
    h = np.zeros((B, H), np.float32)
    em = np.empty((B, T, K), np.float32)
    w_hh_T = np.ascontiguousarray(w_hh.T)
    sig = lambda v: 1.0 / (1.0 + np.exp(-v))
    for t in range(T):
        gh = h @ w_hh_T + b_hh
        gt = gi[:, t]
        r = sig(gt[:, :H] + gh[:, :H])
        z = sig(gt[:, H:2 * H] + gh[:, H:2 * H])
        n = np.tanh(gt[:, 2 * H:] + r * gh[:, 2 * H:])
        h = (1.0 - z) * n + z * h
        em[:, t] = h @ fc_w.T
    em = em + fc_b
    bidx = np.arange(B)
    num = start_trans[tags[:, 0]] + em[bidx, 0, tags[:, 0]]
    num = num + trans[tags[:, :-1], tags[:, 1:]].sum(axis=1)
    num = num + np.take_along_axis(
        em[:, 1:, :], tags[:, 1:, None], axis=2)[:, :, 0].sum(axis=1)
    num = num + end_trans[tags[:, -1]]
    expTr = np.exp(trans).astype(np.float64)
    alpha = (start_trans[None, :] + em[:, 0, :]).astype(np.float64)
    for t in range(1, T):
        m = alpha.max(axis=1)
        alpha = (em[:, t, :] + m[:, None]
                 + np.log(np.exp(alpha - m[:, None]) @ expTr))
    fin = alpha + end_trans[None, :]
    mf = fin.max(axis=1)
    den = mf + np.log(np.exp(fin - mf[:, None]).sum(axis=1))
    return float((den - num).sum())


def kernel(x, tags, emb, w_ih, w_hh, b_ih, b_hh, fc_w, fc_b,
           start_trans, end_trans, trans):
    x = np.asarray(x)
    tags = np.asarray(tags)
    f = lambda a: np.asarray(a, np.float32)
    emb, w_ih, w_hh, b_ih, b_hh, fc_w, fc_b = map(
        f, (emb, w_ih, w_hh, b_ih, b_hh, fc_w, fc_b))
    start_trans, end_trans, trans = map(f, (start_trans, end_trans, trans))
    inputs = dict(x=x, tags=tags, emb=emb, w_ih=w_ih, w_hh=w_hh, b_ih=b_ih,
                  b_hh=b_hh, fc_w=fc_w, fc_b=fc_b, start_trans=start_trans,
                  end_trans=end_trans, trans=trans)
    try:
        nll = _run_device(inputs)
    except Exception:
        import traceback
        traceback.print_exc()
        nll = _host_fallback(**inputs)
    return np.float32(nll)


# revision 27
# speedup vs baseline: 51.5030x; 51.5030x over previous
"""GRU + CRF NLL on 8 NeuronCores, optimized for axon wire cost + fp8 PE.

Per core (8 sequences): fp8 DoubleRow matmuls for the input projection and
the 512-step GRU recurrence; emissions and the CRF forward algorithm (exp
space, sigmoid-division trick, constant normalizer) run on-device
interleaved with the recurrence. Weights ship SHARDED (1/8 per core) and
are reassembled on-device with an AllGather, cutting wire bytes ~8x.
Outputs are tiny per-core CRF scalars; the host finishes the numerator
from tags and sums.
"""
import numpy as np

V, E, H, K, B, T = 32000, 512, 1024, 64, 64, 512
N_CORES = 8
BL = B // N_CORES          # 8 sequences per core
M0 = 4.2                   # CRF constant log-normalizer per step
RENORM = 32                # renorm cadence (steps)
NSLOT = 16                 # mbuf slots
SX, SW, SH = 16.0, 64.0, 16.0   # fp8 scales: x-embed, weights, hidden
SG = SX * SW               # = 1024: psum gate scale


def _to_bf16_u16(a):
    u = np.ascontiguousarray(np.asarray(a, np.float32)).view(np.uint32)
    u = u + 0x7FFF + ((u >> 16) & 1)
    return (u >> 16).astype(np.uint16)


_FP8_LUTS = {}


def _fp8_lut(scale):
    """u16 (bf16 bits) -> u8 bits of float8_e4m3(value*scale); scale is a
    power of two so the fold is exact."""
    if scale not in _FP8_LUTS:
        import ml_dtypes
        allu = np.arange(65536, dtype=np.uint16)
        f = allu.view(ml_dtypes.bfloat16).astype(np.float32) * scale
        f = np.clip(f, -240.0, 240.0)
        with np.errstate(invalid="ignore"):
            _FP8_LUTS[scale] = f.astype(ml_dtypes.float8_e4m3).view(np.uint8)
    return _FP8_LUTS[scale]


def _to_fp8_u8(a, scale):
    u = np.ascontiguousarray(np.asarray(a, np.float32)).view(np.uint32)
    idx = (u + 0x7FFF + ((u >> 16) & 1)) >> 16
    return np.take(_fp8_lut(scale), idx)


def _patch_tile_wait_split():
    from concourse import tile as _tile
    import concourse.mybir as mybir

    cls = None
    for obj in vars(_tile).values():
        if isinstance(obj, type) and "_commit_instruction" in vars(obj):
            cls = obj
            break
    if cls is None or getattr(cls, "_wait_split_patched", False):
        return
    orig = cls._commit_instruction
    ET = mybir.EngineType
    compute = {ET.PE, ET.DVE, ET.Activation, ET.Pool, ET.SP}

    def wrapper(self, inst, lazy_reg_writes=True):
        si = getattr(inst, "sync_info", None)
        eng = getattr(inst, "engine", None)
        if (si is not None and si.on_wait and len(si.on_wait) > 1
                and eng in compute and not isinstance(inst, mybir.InstNoOp)):
            waits = list(si.on_wait)
            for w in waits[:-1]:
                nop = mybir.InstNoOp(
                    name=self.nc.get_next_instruction_name(),
                    engine=eng, bass_nofuse=True,
                    sync_info=mybir.SyncInfo(on_wait=[w], on_update=[]))
                orig(self, nop, lazy_reg_writes)
            inst.sync_info = mybir.SyncInfo(
                on_wait=[waits[-1]], on_update=si.on_update)
        return orig(self, inst, lazy_reg_writes)

    cls._commit_instruction = wrapper

    if "_drain_and_barrier" in vars(cls):
        SC = _tile.ScopedClock

        def patched_db(self, tick_clock, wait_clock):
            drain_inst = self.nc.sync.drain()
            wait_clock.add_sem_waits(
                drain_inst.ins, SC({None: tick_clock.global_clock}))
            d = drain_inst.ins
            si = getattr(d, "sync_info", None)
            if si is not None and si.on_wait and len(si.on_wait) > 1:
                waits = list(si.on_wait)
                d.sync_info = mybir.SyncInfo(
                    on_wait=waits[:1], on_update=si.on_update or [])
                for w in waits[1:]:
                    nop = mybir.InstNoOp(
                        name=self.nc.get_next_instruction_name(),
                        engine=ET.SP, bass_nofuse=True,
                        sync_info=mybir.SyncInfo(on_wait=[w], on_update=[]))
                    self.nc.sync.add_instruction(nop)
            self.nc.all_engine_barrier()
            assert self.sems is not None
            popped = self.nc._tile_sem_poison_stack.pop()
            assert popped is self._sem_poison
            self.nc.clear_and_free_semaphores(
                list(self.sems.allocated().values()))
            self.nc.all_engine_barrier()

        cls._drain_and_barrier = patched_db
    cls._wait_split_patched = True


def _build_nc(t_steps, shard_weights=True):
    import concourse.bass as bass
    import concourse.mybir as mybir
    from concourse.tile import TileContext

    _patch_tile_wait_split()

    f32 = mybir.dt.float32
    bf16 = mybir.dt.bfloat16
    f8 = mybir.dt.float8e4
    AF = mybir.ActivationFunctionType
    ALU = mybir.AluOpType
    DR = mybir.MatmulPerfMode.DoubleRow
    TOK = BL * t_steps
    NTILE = max(1, TOK // 128)
    H3 = 3 * H

    nc = bass.Bass(num_devices=N_CORES)
    # ---- inputs ----
    xe8 = nc.declare_dram_parameter("xe8", [TOK, E], f8, isOutput=False)
    if shard_weights:
        whh_in = nc.declare_dram_parameter("whh8", [H // 8, H3], f8,
                                           isOutput=False)
        wih_in = nc.declare_dram_parameter("wih8", [E // 8, H3], f8,
                                           isOutput=False)
        fcw_in = nc.declare_dram_parameter("fcw8", [H // 8, K], f8,
                                           isOutput=False)
        whh_d = nc.dram_tensor("whh_g", [H, H3], f8, kind="Internal")
        wih_d = nc.dram_tensor("wih_g", [E, H3], f8, kind="Internal")
        fcw_d = nc.dram_tensor("fcw_g", [H, K], f8, kind="Internal")
        whh_s = nc.dram_tensor("whh_s", [H // 8, H3], f8, kind="Internal")
        wih_s = nc.dram_tensor("wih_s", [E // 8, H3], f8, kind="Internal")
        fcw_s = nc.dram_tensor("fcw_s", [H // 8, K], f8, kind="Internal")
    else:
        whh_d = nc.declare_dram_parameter("whh8", [H, H3], f8, isOutput=False)
        wih_d = nc.declare_dram_parameter("wih8", [E, H3], f8, isOutput=False)
        fcw_d = nc.declare_dram_parameter("fcw8", [H, K], f8, isOutput=False)
    biasrow = nc.declare_dram_parameter("biasrow", [1, H3], bf16,
                                        isOutput=False)
    bhnrow = nc.declare_dram_parameter("bhnrow", [1, H], bf16, isOutput=False)
    tagrow = nc.declare_dram_parameter("tagrow", [1, TOK], bf16,
                                       isOutput=False)
    expT_in = nc.declare_dram_parameter("expT", [K, K], bf16, isOutput=False)
    crfb0_in = nc.declare_dram_parameter("crfb0", [K, 1], f32, isOutput=False)
    ncrfb0_in = nc.declare_dram_parameter("ncrfb0", [K, 1], f32,
                                          isOutput=False)
    crfb_in = nc.declare_dram_parameter("crfb", [K, 1], f32, isOutput=False)
    ncrfb_in = nc.declare_dram_parameter("ncrfb", [K, 1], f32, isOutput=False)
    expend_in = nc.declare_dram_parameter("expend", [K, 1], f32,
                                          isOutput=False)
    id128_in = nc.declare_dram_parameter("id128", [128, 128], f8,
                                         isOutput=False)
    id8f8_in = nc.declare_dram_parameter("id8f8", [BL, BL], f8, isOutput=False)
    id8bf_in = nc.declare_dram_parameter("id8bf", [BL, BL], bf16,
                                         isOutput=False)
    id64bf_in = nc.declare_dram_parameter("id64bf", [K, K], bf16,
                                          isOutput=False)
    id16bf_in = nc.declare_dram_parameter("id16bf", [BL, 2 * BL], bf16,
                                          isOutput=False)
    # ---- outputs ----
    emgold_o = nc.declare_dram_parameter("emgold", [K, BL], f32, isOutput=True)
    finsum_o = nc.declare_dram_parameter("finsum", [1, BL], f32, isOutput=True)
    mbuf_o = nc.declare_dram_parameter("mbuf", [BL, NSLOT], f32, isOutput=True)
    gi_d = nc.dram_tensor("gi_scratch", [TOK, H3], bf16, kind="Internal")

    if shard_weights:
        cc_sem = nc.alloc_semaphore("cc_sem")
        cp_sem = nc.alloc_semaphore("cp_sem")
        groups = [list(range(N_CORES))]
        for src, stg in ((whh_in, whh_s), (wih_in, wih_s),
                         (fcw_in, fcw_s)):
            nc.sync.dma_start(out=stg[:], in_=src[:]).then_inc(cp_sem, 16)
        nc.gpsimd.wait_ge(cp_sem, 48)
        for stg, dst in ((whh_s, whh_d), (wih_s, wih_d),
                         (fcw_s, fcw_d)):
            nc.gpsimd.collective_compute(
                "AllGather", mybir.AluOpType.bypass,
                replica_groups=groups,
                ins=[stg[:].opt()], outs=[dst[:].opt()]).then_inc(cc_sem)
        nc.sync.wait_ge(cc_sem, 3)

    with TileContext(nc) as tc:
        with (
            tc.tile_pool(name="wpool", bufs=1) as wpool,
            tc.tile_pool(name="iopool", bufs=4) as iopool,
            tc.tile_pool(name="gates", bufs=2) as gpool,
            tc.tile_pool(name="hpool", bufs=2) as hpool,
            tc.tile_pool(name="crfpool", bufs=2) as cpool,
        ):
            # ---------------- prelude: consts ----------------
            ones_row = wpool.tile([1, 128], bf16, tag="ones")
            nc.vector.memset(ones_row[:], 1.0)
            onescol = wpool.tile([K, 2], bf16, tag="onescol")
            nc.vector.memset(onescol[:], 1.0)
            mbuf_sb = wpool.tile([BL, NSLOT], f32, tag="mbuf")
            nc.vector.memset(mbuf_sb[:], 1.0)
            iota_col = wpool.tile([K, BL], bf16, tag="iotac")
            nc.gpsimd.iota(iota_col[:], pattern=[[0, BL]], base=0,
                           channel_multiplier=1,
                           allow_small_or_imprecise_dtypes=True)

            expT_sb = wpool.tile([K, K], bf16, tag="expT")
            nc.sync.dma_start(out=expT_sb[:], in_=expT_in[:])
            crfb0 = wpool.tile([K, 1], f32, tag="crfb0")
            nc.sync.dma_start(out=crfb0[:], in_=crfb0_in[:])
            ncrfb0 = wpool.tile([K, 1], f32, tag="ncrfb0")
            nc.sync.dma_start(out=ncrfb0[:], in_=ncrfb0_in[:])
            crfb = wpool.tile([K, 1], f32, tag="crfb")
            nc.sync.dma_start(out=crfb[:], in_=crfb_in[:])
            ncrfb = wpool.tile([K, 1], f32, tag="ncrfb")
            nc.sync.dma_start(out=ncrfb[:], in_=ncrfb_in[:])
            expend = wpool.tile([K, 1], f32, tag="expend")
            nc.sync.dma_start(out=expend[:], in_=expend_in[:])
            id128 = wpool.tile([128, 128], f8, tag="id128")
            nc.sync.dma_start(out=id128[:], in_=id128_in[:])
            id8f8 = wpool.tile([BL, BL], f8, tag="id8f8")
            nc.sync.dma_start(out=id8f8[:], in_=id8f8_in[:])
            id8bf = wpool.tile([BL, BL], bf16, tag="id8bf")
            nc.sync.dma_start(out=id8bf[:], in_=id8bf_in[:])
            id64bf = wpool.tile([K, K], bf16, tag="id64bf")
            nc.sync.dma_start(out=id64bf[:], in_=id64bf_in[:])
            id16bf = wpool.tile([BL, 2 * BL], bf16, tag="id16bf")
            nc.sync.dma_start(out=id16bf[:], in_=id16bf_in[:])
            brow_sb = wpool.tile([1, H3], bf16, tag="brow")
            nc.sync.dma_start(out=brow_sb[:], in_=biasrow[:])
            bhn_sb = wpool.tile([1, H], bf16, tag="bhnrow")
            nc.sync.dma_start(out=bhn_sb[:], in_=bhnrow[:])
            tagrow_sb = wpool.tile([1, TOK], bf16, tag="tagrow")
            nc.sync.dma_start(out=tagrow_sb[:], in_=tagrow[:])

            bias_sb = wpool.tile([128, H3], bf16, tag="biasb")
            tags_sb = wpool.tile([K, TOK], bf16, tag="tagsb")
            with tc.tile_pool(name="prelps", bufs=1, space="PSUM") as prelps:
                # broadcast bias row -> [128, 3H]
                for c in range(H3 // 512):
                    bps = prelps.tile([128, 512], f32, tag="bps")
                    nc.tensor.matmul(bps[:], ones_row[:],
                                     brow_sb[:, c * 512:(c + 1) * 512],
                                     start=True, stop=True)
                    nc.scalar.copy(bias_sb[:, c * 512:(c + 1) * 512], bps[:])
                # broadcast tag row -> [64, TOK]
                for c in range((TOK + 511) // 512):
                    w = min(512, TOK - c * 512)
                    tps = prelps.tile([K, 512], f32, tag="tps")
                    nc.tensor.matmul(tps[:, 0:w], ones_row[:, 0:K],
                                     tagrow_sb[:, c * 512:c * 512 + w],
                                     start=True, stop=True)
                    nc.scalar.copy(tags_sb[:, c * 512:c * 512 + w],
                                   tps[:, 0:w])

            # ---------------- weights (after collectives) ----------------
            whh_sb = wpool.tile([128, 8 * H3], f8, tag="whh")
            nc.sync.dma_start(
                out=whh_sb[:].rearrange("p (k g) -> p k g", k=8),
                in_=whh_d.rearrange("(k p) g -> p k g", p=128))
            wih_sb = wpool.tile([128, 4 * H3], f8, tag="wih")
            nc.sync.dma_start(
                out=wih_sb[:].rearrange("p (k g) -> p k g", k=4),
                in_=wih_d.rearrange("(k p) g -> p k g", p=128))
            fcw_sb = wpool.tile([128, 8 * K], f8, tag="fcw")
            nc.sync.dma_start(
                out=fcw_sb[:].rearrange("p (k j) -> p k j", k=8),
                in_=fcw_d.rearrange("(k p) j -> p k j", p=128))

            # ---------------- phase A: gi -> gi_d ----------------
            with (
                tc.tile_pool(name="apool", bufs=3) as apool,
                tc.tile_pool(name="apsA", bufs=2, space="PSUM") as apsA,
                tc.tile_pool(name="apsT", bufs=2, space="PSUM") as apsT,
            ):
                wih3 = wih_sb[:].rearrange("p (k g) -> p k g", k=4)
                for tt in range(NTILE):
                    xs = apool.tile([128, E], f8, tag="xs")
                    nc.sync.dma_start(out=xs[:],
                                      in_=xe8[tt * 128:(tt + 1) * 128, :])
                    xt_ps = apsT.tile([128, 2 * E], f8, tag="xtp")
                    xt_ps2 = xt_ps[:].rearrange("p (e two) -> p e two", two=2)
                    for ec in range(4):
                        nc.tensor.transpose(
                            xt_ps2[:, ec * 128:(ec + 1) * 128, 0:1],
                            xs[:, ec * 128:(ec + 1) * 128], id128[:])
                    xt = apool.tile([128, E], f8, tag="xt")
                    nc.vector.tensor_copy(xt[:], xt_ps2[:, :, 0:1])
                    xt3 = xt[:].rearrange("p (k e) -> p k e", k=4)
                    for gc in range(H3 // 512):
                        ps = apsA.tile([128, 512], f32, tag="aps")
                        for kp in range(2):
                            nc.tensor.matmul(
                                ps[:],
                                xt3[:, 2 * kp:2 * kp + 2, :],
                                wih3[:, 2 * kp:2 * kp + 2,
                                     gc * 512:gc * 512 + 512],
                                start=(kp == 0), stop=(kp == 1),
                                perf_mode=DR)
                        gi_sb = apool.tile([128, 512], bf16, tag="gia")
                        nc.vector.tensor_tensor(
                            gi_sb[:], ps[:],
                            bias_sb[:, gc * 512:gc * 512 + 512], op=ALU.add)
                        nc.sync.dma_start(
                            out=gi_d[tt * 128:(tt + 1) * 128,
                                     gc * 512:gc * 512 + 512],
                            in_=gi_sb[:])

            # ---------------- main loop ----------------
            with (
                tc.tile_pool(name="psG", bufs=1, space="PSUM") as psG,
                tc.tile_pool(name="psHT", bufs=1, space="PSUM") as psHT,
                tc.tile_pool(name="psEC", bufs=2, space="PSUM") as psEC,
                tc.tile_pool(name="psMS", bufs=1, space="PSUM") as psMS,
                tc.tile_pool(name="psMB", bufs=1, space="PSUM") as psMB,
            ):
                whh3 = whh_sb[:].rearrange("p (k g) -> p k g", k=8)
                fcw3 = fcw_sb[:].rearrange("p (k j) -> p k j", k=8)

                ms_tile = psMS.tile([K, 128], f32, tag="ms")
                mb_tile = psMB.tile([BL, 128], bf16, tag="mb")
                tr_ps = psHT.tile([128, 128], bf16, tag="trp")
                hT8 = hpool.tile([128, 128], f8, tag="hT8")
                nc.vector.memset(hT8[:], 0.0)
                h_sb = hpool.tile([BL, H], bf16, tag="h")
                nc.vector.memset(h_sb[:], 0.0)
                aT = cpool.tile([K, BL], bf16, tag="aT")
                acc = wpool.tile([K, BL], f32, tag="acc")
                nc.vector.memset(acc[:], 0.0)

                def em_crf(v, hT8_v, aT_prev):
                    """emissions + CRF for step v (hT8_v = hidden after v)."""
                    hT3 = hT8_v[:].rearrange("p (k b) -> p k b", k=8)
                    ec = psEC.tile([K, 2 * BL], f32, tag="ec")
                    em_ps = ec[:, 0:BL]
                    for kp in range(4):
                        nc.tensor.matmul(
                            em_ps, fcw3[:, 2 * kp:2 * kp + 2, :],
                            hT3[:, 2 * kp:2 * kp + 2, 0:BL],
                            start=(kp == 0), stop=(kp == 3), perf_mode=DR)
                    first = (v == 0)
                    sp = cpool.tile([K, BL], f32, tag="sp")
                    nc.scalar.activation(sp[:], em_ps, AF.Sigmoid,
                                         bias=(crfb0 if first else crfb)[:],
                                         scale=1.0 / SG)
                    sm = cpool.tile([K, BL], f32, tag="sm")
                    nc.scalar.activation(sm[:], em_ps, AF.Sigmoid,
                                         bias=(ncrfb0 if first else ncrfb)[:],
                                         scale=-1.0 / SG)
                    smr = cpool.tile([K, BL], f32, tag="smr")
                    nc.vector.reciprocal(smr[:], sm[:])
                    eem = cpool.tile([K, BL], f32, tag="eem")
                    nc.vector.tensor_tensor(eem[:], sp[:], smr[:],
                                            op=ALU.mult)
                    if first:
                        aT_new = cpool.tile([K, BL], bf16, tag="aT")
                        nc.vector.tensor_copy(aT_new[:], eem[:])
                    else:
                        crf_ps = ec[:, BL:2 * BL]
                        nc.tensor.matmul(crf_ps, expT_sb[:], aT_prev[:],
                                         start=True, stop=True)
                        aT_new = cpool.tile([K, BL], bf16, tag="aT")
                        nc.vector.tensor_tensor(aT_new[:], crf_ps, eem[:],
                                                op=ALU.mult)
                    # gold emission accumulation
                    oh = cpool.tile([K, BL], bf16, tag="oh")
                    nc.vector.tensor_tensor(
                        oh[:], tags_sb[:, v * BL:(v + 1) * BL], iota_col[:],
                        op=ALU.is_equal)
                    gold = cpool.tile([K, BL], f32, tag="gold")
                    nc.vector.tensor_tensor(gold[:], em_ps, oh[:],
                                            op=ALU.mult)
                    nc.vector.tensor_tensor(acc[:], acc[:], gold[:],
                                            op=ALU.add)
                    # periodic renorm
                    if (not first) and v % RENORM == 0:
                        slot = v // RENORM
                        rps = mb_tile[0:BL, 0:K]
                        nc.tensor.transpose(rps, aT_new[:], id64bf[:])
                        m = cpool.tile([BL, 1], f32, tag="m")
                        nc.vector.tensor_reduce(m[:], rps,
                                                axis=mybir.AxisListType.X,
                                                op=ALU.max)
                        rcpf = cpool.tile([BL, 1], f32, tag="rcpf")
                        nc.vector.reciprocal(rcpf[:], m[:])
                        rcp = cpool.tile([BL, 2], bf16, tag="rcp")
                        nc.vector.tensor_copy(rcp[:, 0:1], rcpf[:])
                        nc.vector.tensor_copy(rcp[:, 1:2], rcpf[:])
                        nc.vector.tensor_copy(mbuf_sb[:, slot:slot + 1],
                                              rcp[:, 0:1])
                        rps2 = mb_tile[0:2, K:K + BL]
                        nc.tensor.transpose(rps2, rcp[:], id8bf[:])
                        rrow = cpool.tile([1, BL], bf16, tag="rrow")
                        nc.scalar.copy(rrow[:], rps2[0:1, :])
                        rb_ps = ms_tile[0:K, 0:BL]
                        nc.tensor.matmul(rb_ps, ones_row[:, 0:K], rrow[:],
                                         start=True, stop=True)
                        aT2 = cpool.tile([K, BL], bf16, tag="aT")
                        nc.vector.tensor_tensor(aT2[:], aT_new[:], rb_ps,
                                                op=ALU.mult)
                        aT_new = aT2
                    return aT_new

                for t in range(t_steps):
                    # emissions + CRF for the previous step (hidden ready)
                    if t > 0:
                        aT = em_crf(t - 1, hT8, aT)
                    gi_t = iopool.tile([BL, H3], bf16, tag="gib")
                    nc.sync.dma_start(out=gi_t[:],
                                      in_=gi_d[t * BL:(t + 1) * BL, :])
                    # gh chunks: q0,q1=r; q2,q3=z; q4,q5=n
                    hT3 = hT8[:].rearrange("p (k b) -> p k b", k=8)
                    gq = []
                    for q in range(6):
                        g16 = psG.tile([2 * BL, 512], f32, tag="g%d" % (q % 3))
                        for kp in range(4):
                            nc.tensor.matmul(
                                g16[:], hT3[:, 2 * kp:2 * kp + 2, :],
                                whh3[:, 2 * kp:2 * kp + 2,
                                     q * 512:q * 512 + 512],
                                start=(kp == 0), stop=False, perf_mode=DR)
                        if q < 4:
                            nc.tensor.matmul(
                                g16[:], id16bf[:],
                                gi_t[:, q * 512:q * 512 + 512],
                                start=False, stop=True)
                        else:
                            nc.tensor.matmul(
                                g16[:], ones_row[:, 0:2 * BL],
                                bhn_sb[:, (q - 4) * 512:(q - 4) * 512 + 512],
                                start=False, stop=True)
                        gq.append(g16[0:BL, :])
                    rz = gpool.tile([BL, 2 * H], bf16, tag="rz")
                    for q in range(4):
                        nc.scalar.activation(rz[:, q * 512:q * 512 + 512],
                                             gq[q][:], AF.Sigmoid,
                                             scale=1.0 / SG)
                    n_sb = gpool.tile([BL, H], bf16, tag="n")
                    hn = hpool.tile([BL, H], bf16, tag="h")
                    if t == 0:
                        for jj in range(8):
                            nc.tensor.transpose(
                                tr_ps[:, jj * 16 + 8:jj * 16 + 16],
                                h_sb[:, jj * 128:(jj + 1) * 128], id8bf[:])
                    for c in range(2):
                        sl = slice(c * 512, c * 512 + 512)
                        t1 = gpool.tile([BL, 512], bf16, tag="t1%d" % c)
                        nc.vector.tensor_tensor(t1[:], gq[4 + c][:],
                                                rz[:, sl], op=ALU.mult)
                        t2 = gpool.tile([BL, 512], bf16, tag="t2%d" % c)
                        nc.vector.tensor_tensor(
                            t2[:], t1[:], gi_t[:, 2 * H + c * 512:
                                               2 * H + c * 512 + 512],
                            op=ALU.add)
                        nc.scalar.activation(n_sb[:, sl], t2[:], AF.Tanh,
                                             scale=1.0 / SG)
                        s_c = gpool.tile([BL, 512], bf16, tag="s%d" % c)
                        nc.vector.tensor_tensor(s_c[:], h_sb[:, sl],
                                                n_sb[:, sl], op=ALU.subtract)
                        p_c = gpool.tile([BL, 512], bf16, tag="p%d" % c)
                        nc.vector.tensor_tensor(p_c[:], rz[:, H + sl.start:
                                                           H + sl.stop],
                                                s_c[:], op=ALU.mult)
                        nc.vector.tensor_tensor(hn[:, sl], n_sb[:, sl],
                                                p_c[:], op=ALU.add)
                        for j in range(4):
                            jj = c * 4 + j
                            nc.tensor.transpose(
                                tr_ps[:, jj * 16:jj * 16 + 8],
                                hn[:, jj * 128:(jj + 1) * 128], id8bf[:])
                    h_sb = hn
                    hT8 = hpool.tile([128, 128], f8, tag="hT8")
                    nc.scalar.activation(hT8[:], tr_ps[:], AF.Copy, scale=SH)

                # ---------------- epilogue ----------------
                aT = em_crf(t_steps - 1, hT8, aT)
                fin = cpool.tile([K, BL], bf16, tag="fin")
                nc.vector.tensor_scalar_mul(fin[:], aT[:], expend[:])
                fs_ps = ms_tile[0:2, BL:2 * BL]
                nc.tensor.matmul(fs_ps, onescol[:], fin[:],
                                 start=True, stop=True)
                fs_sb = cpool.tile([1, BL], f32, tag="fssb")
                nc.scalar.copy(fs_sb[:], fs_ps[0:1, :])
                nc.sync.dma_start(out=finsum_o[:], in_=fs_sb[:])
                nc.sync.dma_start(out=emgold_o[:], in_=acc[:])
                nc.sync.dma_start(out=mbuf_o[:], in_=mbuf_sb[:])
    return nc


_NC_CACHE = {}


class _NcShim:
    """Duck-typed stand-in for Bass in run_bass_via_pjrt + lowering: needs
    .m, .to_json_bytes(), .has_collectives, .dbg_addr, .partition_id_tensor.
    """

    def __init__(self, json_bytes):
        import types
        import concourse.mybir as mybir
        self.m = mybir.module_from_json_bytes(json_bytes)
        self._json = json_bytes
        self.has_collectives = True
        self.dbg_addr = None
        self.target_bir_lowering = False
        self.partition_id_tensor = None
        for alloc in self.m.functions[0].allocations:
            if not isinstance(alloc, mybir.MemoryLocationSet):
                continue
            if (alloc.kind == "ExternalInput"
                    and alloc.memorylocations
                    and alloc.memorylocations[0].name == "partition_id"):
                self.partition_id_tensor = types.SimpleNamespace(
                    name="partition_id")

    def to_json_bytes(self):
        return self._json


def _build_version():
    import hashlib
    import inspect
    src = inspect.getsource(_build_nc)
    return hashlib.sha256(src.encode()).hexdigest()[:16]


def _get_nc(t_steps, shard):
    import os
    key = (t_steps, shard)
    if key in _NC_CACHE:
        return _NC_CACHE[key]
    path = os.path.join(_NEFF_CACHE_DIR, "bir_%s_%s_%s.json"
                        % (t_steps, int(shard), _build_version()))
    if os.path.exists(path):
        with open(path, "rb") as f:
            nc = _NcShim(f.read())
    else:
        nc = _build_nc(t_steps, shard)
        try:
            os.makedirs(_NEFF_CACHE_DIR, exist_ok=True)
            data = nc.to_json_bytes()
            tmp = path + ".tmp.%d" % os.getpid()
            with open(tmp, "wb") as f:
                f.write(data)
            os.replace(tmp, path)
        except Exception:
            pass
    _NC_CACHE[key] = nc
    return _NC_CACHE[key]


def _make_in_maps(x, tags, emb, w_ih, w_hh, b_ih, b_hh, fc_w, fc_b,
                  start_trans, end_trans, trans, t_steps=T, shard=True):
    import ml_dtypes
    as8 = lambda u: u.view(ml_dtypes.float8_e4m3)
    asbf = lambda u: u.view(ml_dtypes.bfloat16)

    emb8 = _to_fp8_u8(emb, SX)                      # [V, E]
    whh8 = _to_fp8_u8(np.ascontiguousarray(w_hh.T), SW)   # [H, 3H]
    wih8 = _to_fp8_u8(np.ascontiguousarray(w_ih.T), SW)   # [E, 3H]
    fcw8 = _to_fp8_u8(np.ascontiguousarray(fc_w.T), SW)   # [H, K]
    biasrow = np.concatenate([(b_ih[:2 * H] + b_hh[:2 * H]),
                              b_ih[2 * H:]])[None, :] * SG
    bhnrow = (b_hh[None, 2 * H:] * SG)
    expT = _to_bf16_u16(np.exp(trans))
    crfb0 = (fc_b + start_trans - M0).astype(np.float32)[:, None]
    crfb = (fc_b - M0).astype(np.float32)[:, None]
    expend = np.exp(end_trans).astype(np.float32)[:, None]
    id128 = _to_fp8_u8(np.eye(128, dtype=np.float32), 1.0)
    id8f8 = _to_fp8_u8(np.eye(BL, dtype=np.float32), 1.0)
    id8bf = _to_bf16_u16(np.eye(BL, dtype=np.float32))
    id64bf = _to_bf16_u16(np.eye(K, dtype=np.float32))
    id16bf = _to_bf16_u16(np.concatenate([np.eye(BL), np.eye(BL)],
                                         axis=1).astype(np.float32))

    in_maps = []
    for c in range(N_CORES):
        idxT = x[c * BL:(c + 1) * BL, :t_steps].T.ravel()
        xe8 = np.take(emb8, idxT, axis=0)           # [TOK, E] u8
        tagT = tags[c * BL:(c + 1) * BL, :t_steps].T.reshape(1, -1)
        m = {
            "xe8": as8(xe8),
            "biasrow": asbf(_to_bf16_u16(biasrow)),
            "bhnrow": asbf(_to_bf16_u16(bhnrow)),
            "tagrow": asbf(_to_bf16_u16(tagT.astype(np.float32))),
            "expT": asbf(expT),
            "crfb0": crfb0, "ncrfb0": -crfb0,
            "crfb": crfb, "ncrfb": -crfb,
            "expend": expend,
            "id128": as8(id128), "id8f8": as8(id8f8),
            "id8bf": asbf(id8bf), "id64bf": asbf(id64bf),
            "id16bf": asbf(id16bf),
        }
        if shard:
            m["whh8"] = as8(whh8[c * (H // 8):(c + 1) * (H // 8)])
            m["wih8"] = as8(wih8[c * (E // 8):(c + 1) * (E // 8)])
            m["fcw8"] = as8(fcw8[c * (H // 8):(c + 1) * (H // 8)])
        else:
            m["whh8"] = as8(whh8)
            m["wih8"] = as8(wih8)
            m["fcw8"] = as8(fcw8)
        in_maps.append(m)
    return in_maps


def _finish_host(res, tags, fc_b, start_trans, end_trans, trans, t_steps=T):
    nll = 0.0
    for c in range(N_CORES):
        emgold = np.asarray(res[c]["emgold"], np.float32)    # [K, BL]
        finsum = np.asarray(res[c]["finsum"], np.float32)[0]  # [BL]
        mbuf = np.asarray(res[c]["mbuf"], np.float32)         # [BL, NSLOT]
        tg = tags[c * BL:(c + 1) * BL, :t_steps]
        den = (np.log(finsum) - np.log(mbuf).sum(axis=1)
               + M0 * t_steps)
        emg = emgold.sum(axis=0) / SG + np.take(fc_b, tg).sum(axis=1)
        num = start_trans[tg[:, 0]] + emg
        num += trans[tg[:, :-1], tg[:, 1:]].sum(axis=1)
        num += end_trans[tg[:, -1]]
        nll += float((den - num).sum())
    return nll


_NEFF_CACHE_DIR = "/root/.cache/bass_neff_cache"


def _install_neff_cache():
    """Disk-cache the HLO->NEFF compile (walrus takes 10-80s per fresh
    process otherwise; the stock path has no persistent cache here)."""
    import concourse.bass2jax as b2j
    if getattr(b2j, "_neff_cache_installed", False):
        return
    import hashlib
    import os
    orig = b2j.neuronx_cc_hook

    def cached_hook(code, code_format, platform_version, file_prefix):
        if b"bass_exec" not in code:
            return orig(code, code_format, platform_version, file_prefix)
        key = hashlib.sha256(code).hexdigest()
        path = os.path.join(_NEFF_CACHE_DIR, key + ".bin")
        if os.path.exists(path):
            with open(path, "rb") as f:
                return 0, f.read()
        ret, data = orig(code, code_format, platform_version, file_prefix)
        if ret == 0 and isinstance(data, (bytes, bytearray)):
            os.makedirs(_NEFF_CACHE_DIR, exist_ok=True)
            tmp = path + ".tmp.%d" % os.getpid()
            with open(tmp, "wb") as f:
                f.write(data)
            os.replace(tmp, path)
        return ret, data

    # Second-level cache keyed on the BIR json itself: the serialized HLO
    # bytes are not deterministic across processes, so the whole-result
    # cache above can miss; the BIR is stable and the walrus compile is
    # the expensive part (the per-variant tensor rename is cheap).
    orig_cbk = b2j.compile_bir_kernel

    def cached_cbk(bir_json, tmpdir, neff_name="file.neff"):
        data = bir_json if isinstance(bir_json, bytes) else bir_json.encode()
        key = hashlib.sha256(data).hexdigest()
        path = os.path.join(_NEFF_CACHE_DIR, key + ".neff")
        out_path = os.path.join(tmpdir, neff_name)
        if os.path.exists(path):
            import shutil
            shutil.copy(path, out_path)
            return out_path
        neff_file = orig_cbk(bir_json, tmpdir, neff_name)
        try:
            os.makedirs(_NEFF_CACHE_DIR, exist_ok=True)
            tmp = path + ".tmp.%d" % os.getpid()
            import shutil
            shutil.copy(neff_file, tmp)
            os.replace(tmp, path)
        except Exception:
            pass
        return neff_file

    b2j.compile_bir_kernel = cached_cbk
    b2j.neuronx_cc_hook = cached_hook
    b2j._neff_cache_installed = True


def _run_spmd_fast(nc, in_maps):
    """run_bass_via_pjrt equivalent, but inputs are pre-placed on the mesh
    with sharded device_put (~10 ms/MB) instead of the jit-argument
    transfer path (~75 ms/MB)."""
    import jax
    import concourse.mybir as mybir
    from jax.sharding import Mesh, PartitionSpec, NamedSharding
    from jax.experimental.shard_map import shard_map
    from concourse import bass2jax as b2j

    b2j.install_neuronx_cc_hook()
    partition_name = (nc.partition_id_tensor.name
                      if nc.partition_id_tensor else None)
    in_names, out_names, out_avals, zero_outs = [], [], [], []
    for alloc in nc.m.functions[0].allocations:
        if not isinstance(alloc, mybir.MemoryLocationSet):
            continue
        name = alloc.memorylocations[0].name
        if alloc.kind == "ExternalInput":
            if name != partition_name:
                in_names.append(name)
        elif alloc.kind == "ExternalOutput":
            shape = tuple(alloc.tensor_shape)
            dtype = mybir.dt.np(alloc.dtype)
            out_names.append(name)
            out_avals.append(jax.core.ShapedArray(shape, dtype))
            zero_outs.append(np.zeros(shape, dtype))
    n_params = len(in_names)
    n_outs = len(out_avals)
    all_in_names = list(in_names) + list(out_names)
    if partition_name is not None:
        all_in_names.append(partition_name)

    devices = jax.devices()[:N_CORES]
    mesh = Mesh(np.asarray(devices), ("core",))
    sh = NamedSharding(mesh, PartitionSpec("core"))
    placed = []
    for i, name in enumerate(in_names):
        g = np.concatenate([in_maps[c][name] for c in range(N_CORES)], axis=0)
        placed.append(jax.device_put(g, sh))   # async
    concat_zeros = [np.zeros((N_CORES * z.shape[0], *z.shape[1:]), z.dtype)
                    for z in zero_outs]

    def _body(*args):
        operands = list(args)
        if partition_name is not None:
            operands.append(b2j.partition_id_tensor())
        outs = b2j._bass_exec_p.bind(
            *operands,
            out_avals=tuple(out_avals),
            in_names=tuple(all_in_names),
            out_names=tuple(out_names),
            lowering_input_output_aliases=(),
            sim_require_finite=True,
            sim_require_nnan=True,
            nc=nc,
        )
        return tuple(outs)

    donate = tuple(range(n_params, n_params + n_outs))
    sharded = jax.jit(
        shard_map(_body, mesh=mesh,
                  in_specs=(PartitionSpec("core"),) * (n_params + n_outs),
                  out_specs=(PartitionSpec("core"),) * n_outs,
                  check_rep=False),
        donate_argnums=donate, keep_unused=True)
    out_arrs = sharded(*placed, *concat_zeros)
    return [
        {name: np.asarray(out_arrs[i]).reshape(
            N_CORES, *out_avals[i].shape)[c]
         for i, name in enumerate(out_names)}
        for c in range(N_CORES)
    ]


def _run_device(inputs, t_steps=T, shard=True):
    import sys
    import time as _time
    from concourse.bass_utils import run_bass_kernel_spmd
    _install_neff_cache()
    t0 = _time.time()
    nc = _get_nc(t_steps, shard)
    t1 = _time.time()
    in_maps = _make_in_maps(**inputs, t_steps=t_steps, shard=shard)
    t2 = _time.time()
    res = run_bass_kernel_spmd(nc, in_maps, list(range(N_CORES))).results
    t3 = _time.time()
    out = _finish_host(res, inputs["tags"], inputs["fc_b"],
                       inputs["start_trans"], inputs["end_trans"],
                       inputs["trans"], t_steps=t_steps)
    t4 = _time.time()
    print("[kernel] build=%.0fms host_prep=%.0fms device=%.0fms "
          "finish=%.0fms" % ((t1 - t0) * 1e3, (t2 - t1) * 1e3,
                             (t3 - t2) * 1e3, (t4 - t3) * 1e3),
          file=sys.stderr)
    return out


def _host_fallback(x, tags, emb, w_ih, w_hh, b_ih, b_hh, fc_w, fc_b,
                   start_trans, end_trans, trans):
    xe = emb[x]
    gi = (xe.reshape(-1, E) @ w_ih.T + b_ih).reshape(B, T, 3 * H)
    h = np.zeros((B, H), np.float32)
    em = np.empty((B, T, K), np.float32)
    w_hh_T = np.ascontiguousarray(w_hh.T)
    sig = lambda v: 1.0 / (1.0 + np.exp(-v))
    for t in range(T):
        gh = h @ w_hh_T + b_hh
        gt = gi[:, t]
        r = sig(gt[:, :H] + gh[:, :H])
        z = sig(gt[:, H:2 * H] + gh[:, H:2 * H])
        n = np.tanh(gt[:, 2 * H:] + r * gh[:, 2 * H:])
        h = (1.0 - z) * n + z * h
        em[:, t] = h @ fc_w.T
    em = em + fc_b
    bidx = np.arange(B)
    num = start_trans[tags[:, 0]] + em[bidx, 0, tags[:, 0]]
    num = num + trans[tags[:, :-1], tags[:, 1:]].sum(axis=1)
    num = num + np.take_along_axis(
        em[:, 1:, :], tags[:, 1:, None], axis=2)[:, :, 0].sum(axis=1)
    num = num + end_trans[tags[:, -1]]
    expTr = np.exp(trans).astype(np.float64)
    alpha = (start_trans[None, :] + em[:, 0, :]).astype(np.float64)
    for t in range(1, T):
        m = alpha.max(axis=1)
        alpha = (em[:, t, :] + m[:, None]
                 + np.log(np.exp(alpha - m[:, None]) @ expTr))
    fin = alpha + end_trans[None, :]
    mf = fin.max(axis=1)
    den = mf + np.log(np.exp(fin - mf[:, None]).sum(axis=1))
    return float((den - num).sum())


def kernel(x, tags, emb, w_ih, w_hh, b_ih, b_hh, fc_w, fc_b,
           start_trans, end_trans, trans):
    x = np.asarray(x)
    tags = np.asarray(tags)
    f = lambda a: np.asarray(a, np.float32)
    emb, w_ih, w_hh, b_ih, b_hh, fc_w, fc_b = map(
        f, (emb, w_ih, w_hh, b_ih, b_hh, fc_w, fc_b))
    start_trans, end_trans, trans = map(f, (start_trans, end_trans, trans))
    inputs = dict(x=x, tags=tags, emb=emb, w_ih=w_ih, w_hh=w_hh, b_ih=b_ih,
                  b_hh=b_hh, fc_w=fc_w, fc_b=fc_b, start_trans=start_trans,
                  end_trans=end_trans, trans=trans)
    try:
        nll = _run_device(inputs)
    except Exception:
        import traceback
        traceback.print_exc()
        nll = _host_fallback(**inputs)
    return np.float32(nll)


# revision 30
# speedup vs baseline: 72.2146x; 1.4021x over previous
"""GRU + CRF NLL on 8 NeuronCores, optimized for axon wire cost + fp8 PE.

Per core (8 sequences): fp8 DoubleRow matmuls for the input projection and
the 512-step GRU recurrence; emissions and the CRF forward algorithm (exp
space, sigmoid-division trick, constant normalizer) run on-device
interleaved with the recurrence. Weights ship SHARDED (1/8 per core) and
are reassembled on-device with an AllGather, cutting wire bytes ~8x.
Outputs are tiny per-core CRF scalars; the host finishes the numerator
from tags and sums.
"""
import numpy as np

V, E, H, K, B, T = 32000, 512, 1024, 64, 64, 512
N_CORES = 8
BL = B // N_CORES          # 8 sequences per core
M0 = 4.2                   # CRF constant log-normalizer per step
RENORM = 32                # renorm cadence (steps)
NSLOT = 16                 # mbuf slots
SX, SW, SH = 16.0, 64.0, 16.0   # fp8 scales: x-embed, weights, hidden
SG = SX * SW               # = 1024: psum gate scale


def _to_bf16_u16(a):
    u = np.ascontiguousarray(np.asarray(a, np.float32)).view(np.uint32)
    u = u + 0x7FFF + ((u >> 16) & 1)
    return (u >> 16).astype(np.uint16)


_FP8_LUTS = {}


def _fp8_lut(scale):
    """u16 (bf16 bits) -> u8 bits of float8_e4m3(value*scale); scale is a
    power of two so the fold is exact."""
    if scale not in _FP8_LUTS:
        import ml_dtypes
        allu = np.arange(65536, dtype=np.uint16)
        f = allu.view(ml_dtypes.bfloat16).astype(np.float32) * scale
        f = np.clip(f, -240.0, 240.0)
        with np.errstate(invalid="ignore"):
            _FP8_LUTS[scale] = f.astype(ml_dtypes.float8_e4m3).view(np.uint8)
    return _FP8_LUTS[scale]


def _to_fp8_u8(a, scale):
    u = np.ascontiguousarray(np.asarray(a, np.float32)).view(np.uint32)
    idx = (u + 0x7FFF + ((u >> 16) & 1)) >> 16
    return np.take(_fp8_lut(scale), idx)


def _patch_tile_wait_split():
    from concourse import tile as _tile
    import concourse.mybir as mybir

    cls = None
    for obj in vars(_tile).values():
        if isinstance(obj, type) and "_commit_instruction" in vars(obj):
            cls = obj
            break
    if cls is None or getattr(cls, "_wait_split_patched", False):
        return
    orig = cls._commit_instruction
    ET = mybir.EngineType
    compute = {ET.PE, ET.DVE, ET.Activation, ET.Pool, ET.SP}

    def wrapper(self, inst, lazy_reg_writes=True):
        si = getattr(inst, "sync_info", None)
        eng = getattr(inst, "engine", None)
        if (si is not None and si.on_wait and len(si.on_wait) > 1
                and eng in compute and not isinstance(inst, mybir.InstNoOp)):
            waits = list(si.on_wait)
            for w in waits[:-1]:
                nop = mybir.InstNoOp(
                    name=self.nc.get_next_instruction_name(),
                    engine=eng, bass_nofuse=True,
                    sync_info=mybir.SyncInfo(on_wait=[w], on_update=[]))
                orig(self, nop, lazy_reg_writes)
            inst.sync_info = mybir.SyncInfo(
                on_wait=[waits[-1]], on_update=si.on_update)
        return orig(self, inst, lazy_reg_writes)

    cls._commit_instruction = wrapper

    if "_drain_and_barrier" in vars(cls):
        SC = _tile.ScopedClock

        def patched_db(self, tick_clock, wait_clock):
            drain_inst = self.nc.sync.drain()
            wait_clock.add_sem_waits(
                drain_inst.ins, SC({None: tick_clock.global_clock}))
            d = drain_inst.ins
            si = getattr(d, "sync_info", None)
            if si is not None and si.on_wait and len(si.on_wait) > 1:
                waits = list(si.on_wait)
                d.sync_info = mybir.SyncInfo(
                    on_wait=waits[:1], on_update=si.on_update or [])
                for w in waits[1:]:
                    nop = mybir.InstNoOp(
                        name=self.nc.get_next_instruction_name(),
                        engine=ET.SP, bass_nofuse=True,
                        sync_info=mybir.SyncInfo(on_wait=[w], on_update=[]))
                    self.nc.sync.add_instruction(nop)
            self.nc.all_engine_barrier()
            assert self.sems is not None
            popped = self.nc._tile_sem_poison_stack.pop()
            assert popped is self._sem_poison
            self.nc.clear_and_free_semaphores(
                list(self.sems.allocated().values()))
            self.nc.all_engine_barrier()

        cls._drain_and_barrier = patched_db
    cls._wait_split_patched = True


def _build_nc(t_steps, shard_weights=True):
    import concourse.bass as bass
    import concourse.mybir as mybir
    from concourse.tile import TileContext

    _patch_tile_wait_split()

    f32 = mybir.dt.float32
    bf16 = mybir.dt.bfloat16
    f8 = mybir.dt.float8e4
    AF = mybir.ActivationFunctionType
    ALU = mybir.AluOpType
    DR = mybir.MatmulPerfMode.DoubleRow
    TOK = BL * t_steps
    NTILE = max(1, TOK // 128)
    H3 = 3 * H

    nc = bass.Bass(num_devices=N_CORES)
    # ---- inputs ----
    xe8 = nc.declare_dram_parameter("xe8", [TOK, E], f8, isOutput=False)
    if shard_weights:
        whh_in = nc.declare_dram_parameter("whh8", [H // 8, H3], f8,
                                           isOutput=False)
        wih_in = nc.declare_dram_parameter("wih8", [E // 8, H3], f8,
                                           isOutput=False)
        fcw_in = nc.declare_dram_parameter("fcw8", [H // 8, K], f8,
                                           isOutput=False)
        whh_d = nc.dram_tensor("whh_g", [H, H3], f8, kind="Internal")
        wih_d = nc.dram_tensor("wih_g", [E, H3], f8, kind="Internal")
        fcw_d = nc.dram_tensor("fcw_g", [H, K], f8, kind="Internal")
        whh_s = nc.dram_tensor("whh_s", [H // 8, H3], f8, kind="Internal")
        wih_s = nc.dram_tensor("wih_s", [E // 8, H3], f8, kind="Internal")
        fcw_s = nc.dram_tensor("fcw_s", [H // 8, K], f8, kind="Internal")
    else:
        whh_d = nc.declare_dram_parameter("whh8", [H, H3], f8, isOutput=False)
        wih_d = nc.declare_dram_parameter("wih8", [E, H3], f8, isOutput=False)
        fcw_d = nc.declare_dram_parameter("fcw8", [H, K], f8, isOutput=False)
    biasrow = nc.declare_dram_parameter("biasrow", [1, H3], bf16,
                                        isOutput=False)
    bhnrow = nc.declare_dram_parameter("bhnrow", [1, H], bf16, isOutput=False)
    tagrow = nc.declare_dram_parameter("tagrow", [1, TOK], bf16,
                                       isOutput=False)
    expT_in = nc.declare_dram_parameter("expT", [K, K], bf16, isOutput=False)
    crfb0_in = nc.declare_dram_parameter("crfb0", [K, 1], f32, isOutput=False)
    ncrfb0_in = nc.declare_dram_parameter("ncrfb0", [K, 1], f32,
                                          isOutput=False)
    crfb_in = nc.declare_dram_parameter("crfb", [K, 1], f32, isOutput=False)
    ncrfb_in = nc.declare_dram_parameter("ncrfb", [K, 1], f32, isOutput=False)
    expend_in = nc.declare_dram_parameter("expend", [K, 1], f32,
                                          isOutput=False)
    id128_in = nc.declare_dram_parameter("id128", [128, 128], f8,
                                         isOutput=False)
    id8f8_in = nc.declare_dram_parameter("id8f8", [BL, BL], f8, isOutput=False)
    id8bf_in = nc.declare_dram_parameter("id8bf", [BL, BL], bf16,
                                         isOutput=False)
    id64bf_in = nc.declare_dram_parameter("id64bf", [K, K], bf16,
                                          isOutput=False)
    id16bf_in = nc.declare_dram_parameter("id16bf", [BL, 2 * BL], bf16,
                                          isOutput=False)
    # ---- outputs ----
    emgold_o = nc.declare_dram_parameter("emgold", [K, BL], f32, isOutput=True)
    finsum_o = nc.declare_dram_parameter("finsum", [1, BL], f32, isOutput=True)
    mbuf_o = nc.declare_dram_parameter("mbuf", [BL, NSLOT], f32, isOutput=True)
    gi_d = nc.dram_tensor("gi_scratch", [TOK, H3], bf16, kind="Internal")

    if shard_weights:
        cc_sem = nc.alloc_semaphore("cc_sem")
        cp_sem = nc.alloc_semaphore("cp_sem")
        groups = [list(range(N_CORES))]
        for src, stg in ((whh_in, whh_s), (wih_in, wih_s),
                         (fcw_in, fcw_s)):
            nc.sync.dma_start(out=stg[:], in_=src[:]).then_inc(cp_sem, 16)
        nc.gpsimd.wait_ge(cp_sem, 48)
        for stg, dst in ((whh_s, whh_d), (wih_s, wih_d),
                         (fcw_s, fcw_d)):
            nc.gpsimd.collective_compute(
                "AllGather", mybir.AluOpType.bypass,
                replica_groups=groups,
                ins=[stg[:].opt()], outs=[dst[:].opt()]).then_inc(cc_sem)
        nc.sync.wait_ge(cc_sem, 3)

    with TileContext(nc) as tc:
        with (
            tc.tile_pool(name="wpool", bufs=1) as wpool,
            tc.tile_pool(name="iopool", bufs=4) as iopool,
            tc.tile_pool(name="gates", bufs=2) as gpool,
            tc.tile_pool(name="hpool", bufs=2) as hpool,
            tc.tile_pool(name="crfpool", bufs=2) as cpool,
        ):
            # ---------------- prelude: consts ----------------
            ones_row = wpool.tile([1, 128], bf16, tag="ones")
            nc.vector.memset(ones_row[:], 1.0)
            onescol = wpool.tile([K, 2], bf16, tag="onescol")
            nc.vector.memset(onescol[:], 1.0)
            mbuf_sb = wpool.tile([BL, NSLOT], f32, tag="mbuf")
            nc.vector.memset(mbuf_sb[:], 1.0)
            iota_col = wpool.tile([K, BL], bf16, tag="iotac")
            nc.gpsimd.iota(iota_col[:], pattern=[[0, BL]], base=0,
                           channel_multiplier=1,
                           allow_small_or_imprecise_dtypes=True)

            expT_sb = wpool.tile([K, K], bf16, tag="expT")
            nc.sync.dma_start(out=expT_sb[:], in_=expT_in[:])
            crfb0 = wpool.tile([K, 1], f32, tag="crfb0")
            nc.sync.dma_start(out=crfb0[:], in_=crfb0_in[:])
            ncrfb0 = wpool.tile([K, 1], f32, tag="ncrfb0")
            nc.sync.dma_start(out=ncrfb0[:], in_=ncrfb0_in[:])
            crfb = wpool.tile([K, 1], f32, tag="crfb")
            nc.sync.dma_start(out=crfb[:], in_=crfb_in[:])
            ncrfb = wpool.tile([K, 1], f32, tag="ncrfb")
            nc.sync.dma_start(out=ncrfb[:], in_=ncrfb_in[:])
            expend = wpool.tile([K, 1], f32, tag="expend")
            nc.sync.dma_start(out=expend[:], in_=expend_in[:])
            id128 = wpool.tile([128, 128], f8, tag="id128")
            nc.sync.dma_start(out=id128[:], in_=id128_in[:])
            id8f8 = wpool.tile([BL, BL], f8, tag="id8f8")
            nc.sync.dma_start(out=id8f8[:], in_=id8f8_in[:])
            id8bf = wpool.tile([BL, BL], bf16, tag="id8bf")
            nc.sync.dma_start(out=id8bf[:], in_=id8bf_in[:])
            id64bf = wpool.tile([K, K], bf16, tag="id64bf")
            nc.sync.dma_start(out=id64bf[:], in_=id64bf_in[:])
            id16bf = wpool.tile([BL, 2 * BL], bf16, tag="id16bf")
            nc.sync.dma_start(out=id16bf[:], in_=id16bf_in[:])
            brow_sb = wpool.tile([1, H3], bf16, tag="brow")
            nc.sync.dma_start(out=brow_sb[:], in_=biasrow[:])
            bhn_sb = wpool.tile([1, H], bf16, tag="bhnrow")
            nc.sync.dma_start(out=bhn_sb[:], in_=bhnrow[:])
            tagrow_sb = wpool.tile([1, TOK], bf16, tag="tagrow")
            nc.sync.dma_start(out=tagrow_sb[:], in_=tagrow[:])

            bias_sb = wpool.tile([128, H3], bf16, tag="biasb")
            tags_sb = wpool.tile([K, TOK], bf16, tag="tagsb")
            with tc.tile_pool(name="prelps", bufs=1, space="PSUM") as prelps:
                # broadcast bias row -> [128, 3H]
                for c in range(H3 // 512):
                    bps = prelps.tile([128, 512], f32, tag="bps")
                    nc.tensor.matmul(bps[:], ones_row[:],
                                     brow_sb[:, c * 512:(c + 1) * 512],
                                     start=True, stop=True)
                    nc.scalar.copy(bias_sb[:, c * 512:(c + 1) * 512], bps[:])
                # broadcast tag row -> [64, TOK]
                for c in range((TOK + 511) // 512):
                    w = min(512, TOK - c * 512)
                    tps = prelps.tile([K, 512], f32, tag="tps")
                    nc.tensor.matmul(tps[:, 0:w], ones_row[:, 0:K],
                                     tagrow_sb[:, c * 512:c * 512 + w],
                                     start=True, stop=True)
                    nc.scalar.copy(tags_sb[:, c * 512:c * 512 + w],
                                   tps[:, 0:w])

            # ---------------- weights (after collectives) ----------------
            whh_sb = wpool.tile([128, 8 * H3], f8, tag="whh")
            nc.sync.dma_start(
                out=whh_sb[:].rearrange("p (k g) -> p k g", k=8),
                in_=whh_d.rearrange("(k p) g -> p k g", p=128))
            wih_sb = wpool.tile([128, 4 * H3], f8, tag="wih")
            nc.sync.dma_start(
                out=wih_sb[:].rearrange("p (k g) -> p k g", k=4),
                in_=wih_d.rearrange("(k p) g -> p k g", p=128))
            fcw_sb = wpool.tile([128, 8 * K], f8, tag="fcw")
            nc.sync.dma_start(
                out=fcw_sb[:].rearrange("p (k j) -> p k j", k=8),
                in_=fcw_d.rearrange("(k p) j -> p k j", p=128))

            # ---------------- phase A: gi -> gi_d ----------------
            with (
                tc.tile_pool(name="apool", bufs=3) as apool,
                tc.tile_pool(name="apsA", bufs=2, space="PSUM") as apsA,
                tc.tile_pool(name="apsT", bufs=2, space="PSUM") as apsT,
            ):
                wih3 = wih_sb[:].rearrange("p (k g) -> p k g", k=4)
                for tt in range(NTILE):
                    xs = apool.tile([128, E], f8, tag="xs")
                    nc.sync.dma_start(out=xs[:],
                                      in_=xe8[tt * 128:(tt + 1) * 128, :])
                    xt_ps = apsT.tile([128, 2 * E], f8, tag="xtp")
                    xt_ps2 = xt_ps[:].rearrange("p (e two) -> p e two", two=2)
                    for ec in range(4):
                        nc.tensor.transpose(
                            xt_ps2[:, ec * 128:(ec + 1) * 128, 0:1],
                            xs[:, ec * 128:(ec + 1) * 128], id128[:])
                    xt = apool.tile([128, E], f8, tag="xt")
                    nc.vector.tensor_copy(xt[:], xt_ps2[:, :, 0:1])
                    xt3 = xt[:].rearrange("p (k e) -> p k e", k=4)
                    for gc in range(H3 // 512):
                        ps = apsA.tile([128, 512], f32, tag="aps")
                        for kp in range(2):
                            nc.tensor.matmul(
                                ps[:],
                                xt3[:, 2 * kp:2 * kp + 2, :],
                                wih3[:, 2 * kp:2 * kp + 2,
                                     gc * 512:gc * 512 + 512],
                                start=(kp == 0), stop=(kp == 1),
                                perf_mode=DR)
                        gi_sb = apool.tile([128, 512], bf16, tag="gia")
                        nc.vector.tensor_tensor(
                            gi_sb[:], ps[:],
                            bias_sb[:, gc * 512:gc * 512 + 512], op=ALU.add)
                        nc.sync.dma_start(
                            out=gi_d[tt * 128:(tt + 1) * 128,
                                     gc * 512:gc * 512 + 512],
                            in_=gi_sb[:])

            # ---------------- main loop ----------------
            with (
                tc.tile_pool(name="psG", bufs=1, space="PSUM") as psG,
                tc.tile_pool(name="psHT", bufs=1, space="PSUM") as psHT,
                tc.tile_pool(name="psEC", bufs=2, space="PSUM") as psEC,
                tc.tile_pool(name="psMS", bufs=1, space="PSUM") as psMS,
                tc.tile_pool(name="psMB", bufs=1, space="PSUM") as psMB,
            ):
                whh3 = whh_sb[:].rearrange("p (k g) -> p k g", k=8)
                fcw3 = fcw_sb[:].rearrange("p (k j) -> p k j", k=8)

                ms_tile = psMS.tile([K, 128], f32, tag="ms")
                mb_tile = psMB.tile([BL, 128], bf16, tag="mb")
                tr_ps = psHT.tile([128, 128], bf16, tag="trp")
                hT8 = hpool.tile([128, 128], f8, tag="hT8")
                nc.vector.memset(hT8[:], 0.0)
                h_sb = hpool.tile([BL, H], bf16, tag="h")
                nc.vector.memset(h_sb[:], 0.0)
                aT = cpool.tile([K, BL], bf16, tag="aT")
                acc = wpool.tile([K, BL], f32, tag="acc")
                nc.vector.memset(acc[:], 0.0)

                def em_crf(v, hT8_v, aT_prev):
                    """emissions + CRF for step v (hT8_v = hidden after v)."""
                    hT3 = hT8_v[:].rearrange("p (k b) -> p k b", k=8)
                    ec = psEC.tile([K, 2 * BL], f32, tag="ec")
                    em_ps = ec[:, 0:BL]
                    for kp in range(4):
                        nc.tensor.matmul(
                            em_ps, fcw3[:, 2 * kp:2 * kp + 2, :],
                            hT3[:, 2 * kp:2 * kp + 2, 0:BL],
                            start=(kp == 0), stop=(kp == 3), perf_mode=DR)
                    first = (v == 0)
                    sp = cpool.tile([K, BL], f32, tag="sp")
                    nc.scalar.activation(sp[:], em_ps, AF.Sigmoid,
                                         bias=(crfb0 if first else crfb)[:],
                                         scale=1.0 / SG)
                    sm = cpool.tile([K, BL], f32, tag="sm")
                    nc.scalar.activation(sm[:], em_ps, AF.Sigmoid,
                                         bias=(ncrfb0 if first else ncrfb)[:],
                                         scale=-1.0 / SG)
                    smr = cpool.tile([K, BL], f32, tag="smr")
                    nc.vector.reciprocal(smr[:], sm[:])
                    eem = cpool.tile([K, BL], f32, tag="eem")
                    nc.vector.tensor_tensor(eem[:], sp[:], smr[:],
                                            op=ALU.mult)
                    if first:
                        aT_new = cpool.tile([K, BL], bf16, tag="aT")
                        nc.vector.tensor_copy(aT_new[:], eem[:])
                    else:
                        crf_ps = ec[:, BL:2 * BL]
                        nc.tensor.matmul(crf_ps, expT_sb[:], aT_prev[:],
                                         start=True, stop=True)
                        aT_new = cpool.tile([K, BL], bf16, tag="aT")
                        nc.vector.tensor_tensor(aT_new[:], crf_ps, eem[:],
                                                op=ALU.mult)
                    # gold emission accumulation
                    oh = cpool.tile([K, BL], bf16, tag="oh")
                    nc.vector.tensor_tensor(
                        oh[:], tags_sb[:, v * BL:(v + 1) * BL], iota_col[:],
                        op=ALU.is_equal)
                    gold = cpool.tile([K, BL], f32, tag="gold")
                    nc.vector.tensor_tensor(gold[:], em_ps, oh[:],
                                            op=ALU.mult)
                    nc.vector.tensor_tensor(acc[:], acc[:], gold[:],
                                            op=ALU.add)
                    # periodic renorm
                    if (not first) and v % RENORM == 0:
                        slot = v // RENORM
                        rps = mb_tile[0:BL, 0:K]
                        nc.tensor.transpose(rps, aT_new[:], id64bf[:])
                        m = cpool.tile([BL, 1], f32, tag="m")
                        nc.vector.tensor_reduce(m[:], rps,
                                                axis=mybir.AxisListType.X,
                                                op=ALU.max)
                        rcpf = cpool.tile([BL, 1], f32, tag="rcpf")
                        nc.vector.reciprocal(rcpf[:], m[:])
                        rcp = cpool.tile([BL, 2], bf16, tag="rcp")
                        nc.vector.tensor_copy(rcp[:, 0:1], rcpf[:])
                        nc.vector.tensor_copy(rcp[:, 1:2], rcpf[:])
                        nc.vector.tensor_copy(mbuf_sb[:, slot:slot + 1],
                                              rcp[:, 0:1])
                        rps2 = mb_tile[0:2, K:K + BL]
                        nc.tensor.transpose(rps2, rcp[:], id8bf[:])
                        rrow = cpool.tile([1, BL], bf16, tag="rrow")
                        nc.scalar.copy(rrow[:], rps2[0:1, :])
                        rb_ps = ms_tile[0:K, 0:BL]
                        nc.tensor.matmul(rb_ps, ones_row[:, 0:K], rrow[:],
                                         start=True, stop=True)
                        aT2 = cpool.tile([K, BL], bf16, tag="aT")
                        nc.vector.tensor_tensor(aT2[:], aT_new[:], rb_ps,
                                                op=ALU.mult)
                        aT_new = aT2
                    return aT_new

                for t in range(t_steps):
                    # emissions + CRF for the previous step (hidden ready)
                    if t > 0:
                        aT = em_crf(t - 1, hT8, aT)
                    gi_t = iopool.tile([BL, H3], bf16, tag="gib")
                    nc.sync.dma_start(out=gi_t[:],
                                      in_=gi_d[t * BL:(t + 1) * BL, :])
                    # gh chunks: q0,q1=r; q2,q3=z; q4,q5=n
                    hT3 = hT8[:].rearrange("p (k b) -> p k b", k=8)
                    gq = []
                    for q in range(6):
                        g16 = psG.tile([2 * BL, 512], f32, tag="g%d" % (q % 3))
                        for kp in range(4):
                            nc.tensor.matmul(
                                g16[:], hT3[:, 2 * kp:2 * kp + 2, :],
                                whh3[:, 2 * kp:2 * kp + 2,
                                     q * 512:q * 512 + 512],
                                start=(kp == 0), stop=False, perf_mode=DR)
                        if q < 4:
                            nc.tensor.matmul(
                                g16[:], id16bf[:],
                                gi_t[:, q * 512:q * 512 + 512],
                                start=False, stop=True)
                        else:
                            nc.tensor.matmul(
                                g16[:], ones_row[:, 0:2 * BL],
                                bhn_sb[:, (q - 4) * 512:(q - 4) * 512 + 512],
                                start=False, stop=True)
                        gq.append(g16[0:BL, :])
                    rz = gpool.tile([BL, 2 * H], bf16, tag="rz")
                    for q in range(4):
                        nc.scalar.activation(rz[:, q * 512:q * 512 + 512],
                                             gq[q][:], AF.Sigmoid,
                                             scale=1.0 / SG)
                    n_sb = gpool.tile([BL, H], bf16, tag="n")
                    hn = hpool.tile([BL, H], bf16, tag="h")
                    if t == 0:
                        for jj in range(8):
                            nc.tensor.transpose(
                                tr_ps[:, jj * 16 + 8:jj * 16 + 16],
                                h_sb[:, jj * 128:(jj + 1) * 128], id8bf[:])
                    for c in range(2):
                        sl = slice(c * 512, c * 512 + 512)
                        t1 = gpool.tile([BL, 512], bf16, tag="t1%d" % c)
                        nc.vector.tensor_tensor(t1[:], gq[4 + c][:],
                                                rz[:, sl], op=ALU.mult)
                        t2 = gpool.tile([BL, 512], bf16, tag="t2%d" % c)
                        nc.vector.tensor_tensor(
                            t2[:], t1[:], gi_t[:, 2 * H + c * 512:
                                               2 * H + c * 512 + 512],
                            op=ALU.add)
                        nc.scalar.activation(n_sb[:, sl], t2[:], AF.Tanh,
                                             scale=1.0 / SG)
                        s_c = gpool.tile([BL, 512], bf16, tag="s%d" % c)
                        nc.vector.tensor_tensor(s_c[:], h_sb[:, sl],
                                                n_sb[:, sl], op=ALU.subtract)
                        p_c = gpool.tile([BL, 512], bf16, tag="p%d" % c)
                        nc.vector.tensor_tensor(p_c[:], rz[:, H + sl.start:
                                                           H + sl.stop],
                                                s_c[:], op=ALU.mult)
                        nc.vector.tensor_tensor(hn[:, sl], n_sb[:, sl],
                                                p_c[:], op=ALU.add)
                        for j in range(4):
                            jj = c * 4 + j
                            nc.tensor.transpose(
                                tr_ps[:, jj * 16:jj * 16 + 8],
                                hn[:, jj * 128:(jj + 1) * 128], id8bf[:])
                    h_sb = hn
                    hT8 = hpool.tile([128, 128], f8, tag="hT8")
                    nc.scalar.activation(hT8[:], tr_ps[:], AF.Copy, scale=SH)

                # ---------------- epilogue ----------------
                aT = em_crf(t_steps - 1, hT8, aT)
                fin = cpool.tile([K, BL], bf16, tag="fin")
                nc.vector.tensor_scalar_mul(fin[:], aT[:], expend[:])
                fs_ps = ms_tile[0:2, BL:2 * BL]
                nc.tensor.matmul(fs_ps, onescol[:], fin[:],
                                 start=True, stop=True)
                fs_sb = cpool.tile([1, BL], f32, tag="fssb")
                nc.scalar.copy(fs_sb[:], fs_ps[0:1, :])
                nc.sync.dma_start(out=finsum_o[:], in_=fs_sb[:])
                nc.sync.dma_start(out=emgold_o[:], in_=acc[:])
                nc.sync.dma_start(out=mbuf_o[:], in_=mbuf_sb[:])
    return nc


_NC_CACHE = {}


class _NcShim:
    """Duck-typed stand-in for Bass in run_bass_via_pjrt + lowering: needs
    .m, .to_json_bytes(), .has_collectives, .dbg_addr, .partition_id_tensor.
    """

    def __init__(self, json_bytes):
        import types
        import concourse.mybir as mybir
        self.m = mybir.module_from_json_bytes(json_bytes)
        self._json = json_bytes
        self.has_collectives = True
        self.dbg_addr = None
        self.target_bir_lowering = False
        self.partition_id_tensor = None
        for alloc in self.m.functions[0].allocations:
            if not isinstance(alloc, mybir.MemoryLocationSet):
                continue
            if (alloc.kind == "ExternalInput"
                    and alloc.memorylocations
                    and alloc.memorylocations[0].name == "partition_id"):
                self.partition_id_tensor = types.SimpleNamespace(
                    name="partition_id")

    def to_json_bytes(self):
        return self._json


def _build_version():
    import hashlib
    import inspect
    src = inspect.getsource(_build_nc)
    return hashlib.sha256(src.encode()).hexdigest()[:16]


class _NcMeta:
    """Metadata-only stand-in: enough for a run_bass_via_pjrt replica and
    the bass_exec lowering, with no module parse (to_json_bytes from disk).
    """

    def __init__(self, json_path, meta):
        import types
        self._json_path = json_path
        self._json = None
        self.meta = meta
        self.m = types.SimpleNamespace(arch=meta["arch"])
        self.has_collectives = True
        self.dbg_addr = None
        self.target_bir_lowering = False
        self.partition_id_tensor = (
            types.SimpleNamespace(name="partition_id")
            if meta["has_partition_id"] else None)

    def to_json_bytes(self):
        if self._json is None:
            with open(self._json_path, "rb") as f:
                self._json = f.read()
        return self._json


def _extract_meta(m):
    import concourse.mybir as mybir
    meta = {"arch": m.arch, "has_partition_id": False,
            "inputs": [], "outputs": []}
    for alloc in m.functions[0].allocations:
        if not isinstance(alloc, mybir.MemoryLocationSet):
            continue
        name = alloc.memorylocations[0].name
        if alloc.kind == "ExternalInput":
            if name == "partition_id":
                meta["has_partition_id"] = True
            else:
                meta["inputs"].append(name)
        elif alloc.kind == "ExternalOutput":
            meta["outputs"].append(
                (name, list(alloc.tensor_shape),
                 np.dtype(mybir.dt.np(alloc.dtype)).str))
    return meta


def _get_nc(t_steps, shard):
    import json
    import os
    key = (t_steps, shard)
    if key in _NC_CACHE:
        return _NC_CACHE[key]
    path = os.path.join(_NEFF_CACHE_DIR, "bir_%s_%s_%s.json"
                        % (t_steps, int(shard), _build_version()))
    mpath = path + ".meta"
    if os.path.exists(path) and os.path.exists(mpath):
        with open(mpath) as f:
            nc = _NcMeta(path, json.load(f))
    else:
        if os.path.exists(path):
            with open(path, "rb") as f:
                nc = _NcShim(f.read())
        else:
            nc = _build_nc(t_steps, shard)
            try:
                os.makedirs(_NEFF_CACHE_DIR, exist_ok=True)
                data = nc.to_json_bytes()
                tmp = path + ".tmp.%d" % os.getpid()
                with open(tmp, "wb") as f:
                    f.write(data)
                os.replace(tmp, path)
            except Exception:
                pass
        try:
            meta = _extract_meta(nc.m)
            tmp = mpath + ".tmp.%d" % os.getpid()
            with open(tmp, "w") as f:
                json.dump(meta, f)
            os.replace(tmp, mpath)
        except Exception:
            pass
    _NC_CACHE[key] = nc
    return _NC_CACHE[key]


def _make_in_maps(x, tags, emb, w_ih, w_hh, b_ih, b_hh, fc_w, fc_b,
                  start_trans, end_trans, trans, t_steps=T, shard=True):
    import ml_dtypes
    as8 = lambda u: u.view(ml_dtypes.float8_e4m3)
    asbf = lambda u: u.view(ml_dtypes.bfloat16)

    emb8 = _to_fp8_u8(emb, SX)                      # [V, E]
    whh8 = _to_fp8_u8(np.ascontiguousarray(w_hh.T), SW)   # [H, 3H]
    wih8 = _to_fp8_u8(np.ascontiguousarray(w_ih.T), SW)   # [E, 3H]
    fcw8 = _to_fp8_u8(np.ascontiguousarray(fc_w.T), SW)   # [H, K]
    biasrow = np.concatenate([(b_ih[:2 * H] + b_hh[:2 * H]),
                              b_ih[2 * H:]])[None, :] * SG
    bhnrow = (b_hh[None, 2 * H:] * SG)
    expT = _to_bf16_u16(np.exp(trans))
    crfb0 = (fc_b + start_trans - M0).astype(np.float32)[:, None]
    crfb = (fc_b - M0).astype(np.float32)[:, None]
    expend = np.exp(end_trans).astype(np.float32)[:, None]
    id128 = _to_fp8_u8(np.eye(128, dtype=np.float32), 1.0)
    id8f8 = _to_fp8_u8(np.eye(BL, dtype=np.float32), 1.0)
    id8bf = _to_bf16_u16(np.eye(BL, dtype=np.float32))
    id64bf = _to_bf16_u16(np.eye(K, dtype=np.float32))
    id16bf = _to_bf16_u16(np.concatenate([np.eye(BL), np.eye(BL)],
                                         axis=1).astype(np.float32))

    in_maps = []
    for c in range(N_CORES):
        idxT = x[c * BL:(c + 1) * BL, :t_steps].T.ravel()
        xe8 = np.take(emb8, idxT, axis=0)           # [TOK, E] u8
        tagT = tags[c * BL:(c + 1) * BL, :t_steps].T.reshape(1, -1)
        m = {
            "xe8": as8(xe8),
            "biasrow": asbf(_to_bf16_u16(biasrow)),
            "bhnrow": asbf(_to_bf16_u16(bhnrow)),
            "tagrow": asbf(_to_bf16_u16(tagT.astype(np.float32))),
            "expT": asbf(expT),
            "crfb0": crfb0, "ncrfb0": -crfb0,
            "crfb": crfb, "ncrfb": -crfb,
            "expend": expend,
            "id128": as8(id128), "id8f8": as8(id8f8),
            "id8bf": asbf(id8bf), "id64bf": asbf(id64bf),
            "id16bf": asbf(id16bf),
        }
        if shard:
            m["whh8"] = as8(whh8[c * (H // 8):(c + 1) * (H // 8)])
            m["wih8"] = as8(wih8[c * (E // 8):(c + 1) * (E // 8)])
            m["fcw8"] = as8(fcw8[c * (H // 8):(c + 1) * (H // 8)])
        else:
            m["whh8"] = as8(whh8)
            m["wih8"] = as8(wih8)
            m["fcw8"] = as8(fcw8)
        in_maps.append(m)
    return in_maps


def _finish_host(res, tags, fc_b, start_trans, end_trans, trans, t_steps=T):
    nll = 0.0
    for c in range(N_CORES):
        emgold = np.asarray(res[c]["emgold"], np.float32)    # [K, BL]
        finsum = np.asarray(res[c]["finsum"], np.float32)[0]  # [BL]
        mbuf = np.asarray(res[c]["mbuf"], np.float32)         # [BL, NSLOT]
        tg = tags[c * BL:(c + 1) * BL, :t_steps]
        den = (np.log(finsum) - np.log(mbuf).sum(axis=1)
               + M0 * t_steps)
        emg = emgold.sum(axis=0) / SG + np.take(fc_b, tg).sum(axis=1)
        num = start_trans[tg[:, 0]] + emg
        num += trans[tg[:, :-1], tg[:, 1:]].sum(axis=1)
        num += end_trans[tg[:, -1]]
        nll += float((den - num).sum())
    return nll


_NEFF_CACHE_DIR = "/root/.cache/bass_neff_cache"


def _install_neff_cache():
    """Disk-cache the HLO->NEFF compile (walrus takes 10-80s per fresh
    process otherwise; the stock path has no persistent cache here)."""
    import concourse.bass2jax as b2j
    if getattr(b2j, "_neff_cache_installed", False):
        return
    import hashlib
    import os
    orig = b2j.neuronx_cc_hook

    def cached_hook(code, code_format, platform_version, file_prefix):
        if b"bass_exec" not in code:
            return orig(code, code_format, platform_version, file_prefix)
        key = hashlib.sha256(code).hexdigest()
        path = os.path.join(_NEFF_CACHE_DIR, key + ".bin")
        if os.path.exists(path):
            with open(path, "rb") as f:
                return 0, f.read()
        ret, data = orig(code, code_format, platform_version, file_prefix)
        if ret == 0 and isinstance(data, (bytes, bytearray)):
            os.makedirs(_NEFF_CACHE_DIR, exist_ok=True)
            tmp = path + ".tmp.%d" % os.getpid()
            with open(tmp, "wb") as f:
                f.write(data)
            os.replace(tmp, path)
        return ret, data

    # Second-level cache keyed on the BIR json itself: the serialized HLO
    # bytes are not deterministic across processes, so the whole-result
    # cache above can miss; the BIR is stable and the walrus compile is
    # the expensive part (the per-variant tensor rename is cheap).
    orig_cbk = b2j.compile_bir_kernel

    def cached_cbk(bir_json, tmpdir, neff_name="file.neff"):
        data = bir_json if isinstance(bir_json, bytes) else bir_json.encode()
        key = hashlib.sha256(data).hexdigest()
        path = os.path.join(_NEFF_CACHE_DIR, key + ".neff")
        out_path = os.path.join(tmpdir, neff_name)
        if os.path.exists(path):
            import shutil
            shutil.copy(path, out_path)
            return out_path
        neff_file = orig_cbk(bir_json, tmpdir, neff_name)
        try:
            os.makedirs(_NEFF_CACHE_DIR, exist_ok=True)
            tmp = path + ".tmp.%d" % os.getpid()
            import shutil
            shutil.copy(neff_file, tmp)
            os.replace(tmp, path)
        except Exception:
            pass
        return neff_file

    b2j.compile_bir_kernel = cached_cbk
    b2j.neuronx_cc_hook = cached_hook
    b2j._neff_cache_installed = True


def _run_spmd_fast(nc, in_maps):
    """run_bass_via_pjrt equivalent, but inputs are pre-placed on the mesh
    with sharded device_put (~10 ms/MB) instead of the jit-argument
    transfer path (~75 ms/MB)."""
    import jax
    import concourse.mybir as mybir
    from jax.sharding import Mesh, PartitionSpec, NamedSharding
    from jax.experimental.shard_map import shard_map
    from concourse import bass2jax as b2j

    b2j.install_neuronx_cc_hook()
    partition_name = (nc.partition_id_tensor.name
                      if nc.partition_id_tensor else None)
    in_names, out_names, out_avals, zero_outs = [], [], [], []
    for alloc in nc.m.functions[0].allocations:
        if not isinstance(alloc, mybir.MemoryLocationSet):
            continue
        name = alloc.memorylocations[0].name
        if alloc.kind == "ExternalInput":
            if name != partition_name:
                in_names.append(name)
        elif alloc.kind == "ExternalOutput":
            shape = tuple(alloc.tensor_shape)
            dtype = mybir.dt.np(alloc.dtype)
            out_names.append(name)
            out_avals.append(jax.core.ShapedArray(shape, dtype))
            zero_outs.append(np.zeros(shape, dtype))
    n_params = len(in_names)
    n_outs = len(out_avals)
    all_in_names = list(in_names) + list(out_names)
    if partition_name is not None:
        all_in_names.append(partition_name)

    devices = jax.devices()[:N_CORES]
    mesh = Mesh(np.asarray(devices), ("core",))
    sh = NamedSharding(mesh, PartitionSpec("core"))
    placed = []
    for i, name in enumerate(in_names):
        g = np.concatenate([in_maps[c][name] for c in range(N_CORES)], axis=0)
        placed.append(jax.device_put(g, sh))   # async
    concat_zeros = [np.zeros((N_CORES * z.shape[0], *z.shape[1:]), z.dtype)
                    for z in zero_outs]

    def _body(*args):
        operands = list(args)
        if partition_name is not None:
            operands.append(b2j.partition_id_tensor())
        outs = b2j._bass_exec_p.bind(
            *operands,
            out_avals=tuple(out_avals),
            in_names=tuple(all_in_names),
            out_names=tuple(out_names),
            lowering_input_output_aliases=(),
            sim_require_finite=True,
            sim_require_nnan=True,
            nc=nc,
        )
        return tuple(outs)

    donate = tuple(range(n_params, n_params + n_outs))
    sharded = jax.jit(
        shard_map(_body, mesh=mesh,
                  in_specs=(PartitionSpec("core"),) * (n_params + n_outs),
                  out_specs=(PartitionSpec("core"),) * n_outs,
                  check_rep=False),
        donate_argnums=donate, keep_unused=True)
    out_arrs = sharded(*placed, *concat_zeros)
    return [
        {name: np.asarray(out_arrs[i]).reshape(
            N_CORES, *out_avals[i].shape)[c]
         for i, name in enumerate(out_names)}
        for c in range(N_CORES)
    ]


def _run_spmd_meta(nc, in_maps):
    """run_bass_via_pjrt's multi-core branch, fed from sidecar metadata so
    no module parse is needed. The jit structure matches the stock path
    byte-for-byte so existing compiled-executable caches stay valid."""
    import jax
    from jax.sharding import Mesh, PartitionSpec
    from jax.experimental.shard_map import shard_map
    from concourse import bass2jax as b2j

    b2j.install_neuronx_cc_hook()
    meta = nc.meta
    in_names = list(meta["inputs"])
    out_names = [o[0] for o in meta["outputs"]]
    out_avals = [jax.core.ShapedArray(tuple(shape), np.dtype(ds))
                 for _, shape, ds in meta["outputs"]]
    zero_outs = [np.zeros(tuple(shape), np.dtype(ds))
                 for _, shape, ds in meta["outputs"]]
    n_params = len(in_names)
    n_outs = len(out_avals)
    in_names.extend(out_names)
    partition_name = (nc.partition_id_tensor.name
                      if nc.partition_id_tensor else None)
    if partition_name is not None:
        in_names.append(partition_name)

    def _per_core_inputs(in_map):
        return [np.asarray(in_map[name]) for name in in_names[:n_params]]

    donate = tuple(range(n_params, n_params + n_outs))

    def _body(*args):
        operands = list(args)
        if partition_name is not None:
            operands.append(b2j.partition_id_tensor())
        outs = b2j._bass_exec_p.bind(
            *operands,
            out_avals=tuple(out_avals),
            in_names=tuple(in_names),
            out_names=tuple(out_names),
            lowering_input_output_aliases=(),
            sim_require_finite=True,
            sim_require_nnan=True,
            nc=nc,
        )
        return tuple(outs)

    devices = jax.devices()[:N_CORES]
    mesh = Mesh(np.asarray(devices), ("core",))
    in_specs = (PartitionSpec("core"),) * (n_params + n_outs)
    out_specs = (PartitionSpec("core"),) * len(out_names)
    sharded = jax.jit(
        shard_map(_body, mesh=mesh, in_specs=in_specs, out_specs=out_specs,
                  check_rep=False),
        donate_argnums=donate, keep_unused=True)
    per_core = [_per_core_inputs(m) for m in in_maps]
    concat_in = [np.concatenate([per_core[c][i] for c in range(N_CORES)],
                                axis=0) for i in range(n_params)]
    concat_zeros = [np.zeros((N_CORES * z.shape[0], *z.shape[1:]), z.dtype)
                    for z in zero_outs]
    out_arrs = sharded(*concat_in, *concat_zeros)
    return [
        {name: np.asarray(out_arrs[i]).reshape(
            N_CORES, *out_avals[i].shape)[c]
         for i, name in enumerate(out_names)}
        for c in range(N_CORES)
    ]


def _run_device(inputs, t_steps=T, shard=True):
    import sys
    import time as _time
    from concourse.bass_utils import run_bass_kernel_spmd
    _install_neff_cache()
    t0 = _time.time()
    nc = _get_nc(t_steps, shard)
    t1 = _time.time()
    in_maps = _make_in_maps(**inputs, t_steps=t_steps, shard=shard)
    t2 = _time.time()
    if isinstance(nc, _NcMeta):
        res = _run_spmd_meta(nc, in_maps)
    else:
        res = run_bass_kernel_spmd(nc, in_maps,
                                   list(range(N_CORES))).results
    t3 = _time.time()
    out = _finish_host(res, inputs["tags"], inputs["fc_b"],
                       inputs["start_trans"], inputs["end_trans"],
                       inputs["trans"], t_steps=t_steps)
    t4 = _time.time()
    print("[kernel] build=%.0fms host_prep=%.0fms device=%.0fms "
          "finish=%.0fms" % ((t1 - t0) * 1e3, (t2 - t1) * 1e3,
                             (t3 - t2) * 1e3, (t4 - t3) * 1e3),
          file=sys.stderr)
    return out


def _host_fallback(x, tags, emb, w_ih, w_hh, b_ih, b_hh, fc_w, fc_b,
                   start_trans, end_trans, trans):
    xe = emb[x]
    gi = (xe.reshape(-1, E) @ w_ih.T + b_ih).reshape(B, T, 3 * H)
    h = np.zeros((B, H), np.float32)
    em = np.empty((B, T, K), np.float32)
    w_hh_T = np.ascontiguousarray(w_hh.T)
    sig = lambda v: 1.0 / (1.0 + np.exp(-v))
    for t in range(T):
        gh = h @ w_hh_T + b_hh
        gt = gi[:, t]
        r = sig(gt[:, :H] + gh[:, :H])
        z = sig(gt[:, H:2 * H] + gh[:, H:2 * H])
        n = np.tanh(gt[:, 2 * H:] + r * gh[:, 2 * H:])
        h = (1.0 - z) * n + z * h
        em[:, t] = h @ fc_w.T
    em = em + fc_b
    bidx = np.arange(B)
    num = start_trans[tags[:, 0]] + em[bidx, 0, tags[:, 0]]
    num = num + trans[tags[:, :-1], tags[:, 1:]].sum(axis=1)
    num = num + np.take_along_axis(
        em[:, 1:, :], tags[:, 1:, None], axis=2)[:, :, 0].sum(axis=1)
    num = num + end_trans[tags[:, -1]]
    expTr = np.exp(trans).astype(np.float64)
    alpha = (start_trans[None, :] + em[:, 0, :]).astype(np.float64)
    for t in range(1, T):
        m = alpha.max(axis=1)
        alpha = (em[:, t, :] + m[:, None]
                 + np.log(np.exp(alpha - m[:, None]) @ expTr))
    fin = alpha + end_trans[None, :]
    mf = fin.max(axis=1)
    den = mf + np.log(np.exp(fin - mf[:, None]).sum(axis=1))
    return float((den - num).sum())


def kernel(x, tags, emb, w_ih, w_hh, b_ih, b_hh, fc_w, fc_b,
           start_trans, end_trans, trans):
    x = np.asarray(x)
    tags = np.asarray(tags)
    f = lambda a: np.asarray(a, np.float32)
    emb, w_ih, w_hh, b_ih, b_hh, fc_w, fc_b = map(
        f, (emb, w_ih, w_hh, b_ih, b_hh, fc_w, fc_b))
    start_trans, end_trans, trans = map(f, (start_trans, end_trans, trans))
    inputs = dict(x=x, tags=tags, emb=emb, w_ih=w_ih, w_hh=w_hh, b_ih=b_ih,
                  b_hh=b_hh, fc_w=fc_w, fc_b=fc_b, start_trans=start_trans,
                  end_trans=end_trans, trans=trans)
    try:
        nll = _run_device(inputs)
    except Exception:
        import traceback
        traceback.print_exc()
        nll = _host_fallback(**inputs)
    return np.float32(nll)
